# revision 13
# baseline (speedup 1.0000x reference)
"""Trainium2 Bass kernel for CorrelatedGraphConv.

Reference computation (per batch b, N=100 rows, D=1024, L=2000 labels):
    adj   = (graph != 0)
    lin   = x + x@W0.T + x@W1.T + sum_j bias[graph[:, j]]
    a     = x@Wa.T + ba ; bvec = x@Wb.T + bb
    alpha = relu(a @ bvec.T)
    alpha = softmax(adj @ alpha, axis=0)   # over rows i
    out   = alpha @ lin

Strategy: data-parallel over batch across 8 cores (2 batches/core).
The label-gather `sum_j bias[g[i,j]]` is computed as `C @ bias` where
C[i, l] = #{j : g[i,j] == l} is built on-chip with gpsimd.local_scatter
(per-partition indexed scatter); duplicate labels within a row are
pre-combined on DVE (count via self-equality matmul-free compare, only
the first occurrence scatters the total count).
"""

import numpy as np

import concourse.bass as bass
import concourse.mybir as mybir
import concourse.tile as tile
from concourse import bacc, library_config

F32 = mybir.dt.float32
F16 = mybir.dt.float16
I16 = mybir.dt.int16

B, N, D, L = 16, 100, 1024, 2000
NCORES = 8
BPC = B // NCORES          # batches per core
R = BPC * N                # rows per core (200)
DT = D // 128              # 8 d-tiles
LT_TILES = (L + 127) // 128  # 16 label tiles (last is 80)

_CACHE = {}


def _bcast3(ap, mid, inner, mode):
    """[P, F] AP -> [P, mid, inner] broadcast view.

    mode 'j':  out[p, a, b] = ap[p, a]   (inner broadcast)
    mode 'jp': out[p, a, b] = ap[p, b]   (middle broadcast)
    """
    (pstep, pcount), (fstep, fcount) = ap.ap[0], ap.ap[1]
    if mode == "j":
        assert fcount == mid
        new = [[pstep, pcount], [fstep, mid], [0, inner]]
    else:
        assert fcount == inner
        new = [[pstep, pcount], [0, mid], [fstep, inner]]
    return bass.AP(tensor=ap.tensor, offset=ap.offset, ap=new)


def _pbcast(ap, p):
    """[1, ...] AP -> [p, ...] partition-broadcast view."""
    new = [[0, p]] + [list(d) for d in ap.ap[1:]]
    return bass.AP(tensor=ap.tensor, offset=ap.offset, ap=new)


def _build_program():
    nc = bacc.Bacc("TRN2", target_bir_lowering=False, debug=False,
                   num_devices=NCORES)

    x_d = nc.declare_dram_parameter("x", [R, D], F32, isOutput=False)
    id32_d = nc.declare_dram_parameter("id32", [128, 128], F32, isOutput=False)
    id16_d = nc.declare_dram_parameter("id16", [128, 128], F16, isOutput=False)
    g_d = nc.declare_dram_parameter("g16", [R, N], F16, isOutput=False)
    lt_d = nc.declare_dram_parameter("ltmask", [N * N], F16, isOutput=False)
    wct_d = nc.declare_dram_parameter("wct", [D, D], F32, isOutput=False)
    wat_d = nc.declare_dram_parameter("wat", [D, D], F32, isOutput=False)
    wbt_d = nc.declare_dram_parameter("wbt", [D, D], F32, isOutput=False)
    bias_d = nc.declare_dram_parameter("bias", [L, D], F32, isOutput=False)
    ba_d = nc.declare_dram_parameter("ba", [D], F32, isOutput=False)
    bb_d = nc.declare_dram_parameter("bb", [D], F32, isOutput=False)
    out_d = nc.declare_dram_parameter("out", [R, D], F32, isOutput=True)

    with tile.TileContext(nc) as tc:
        _emit(tc, x_d, id32_d, id16_d, g_d, lt_d, wct_d, wat_d, wbt_d,
              bias_d, ba_d, bb_d, out_d)
    nc.compile()
    return nc


def _emit(tc, x_d, id32_d, id16_d, g_d, lt_d, wct_d, wat_d, wbt_d, bias_d,
          ba_d, bb_d, out_d):
    nc = tc.nc
    import contextlib

    ctx = contextlib.ExitStack()
    with ctx:
        const = ctx.enter_context(tc.tile_pool(name="const", bufs=1))
        gpool = ctx.enter_context(tc.tile_pool(name="gtiles", bufs=1))
        xpool = ctx.enter_context(tc.tile_pool(name="xtiles", bufs=1))
        xt = ctx.enter_context(tc.tile_pool(name="xt", bufs=1))
        abp = ctx.enter_context(tc.tile_pool(name="abt", bufs=1))
        linp = ctx.enter_context(tc.tile_pool(name="lin", bufs=1))
        cb = ctx.enter_context(tc.tile_pool(name="cbuild", bufs=2))
        cpool = ctx.enter_context(tc.tile_pool(name="cmat", bufs=1))
        ctp = ctx.enter_context(tc.tile_pool(name="ctmat", bufs=1))
        wstream = ctx.enter_context(tc.tile_pool(name="wstream", bufs=4))
        bstream = ctx.enter_context(tc.tile_pool(name="bstream", bufs=4))
        small = ctx.enter_context(tc.tile_pool(name="small", bufs=4))
        outp = ctx.enter_context(tc.tile_pool(name="outs", bufs=2))
        pst = ctx.enter_context(tc.tile_pool(name="pst", bufs=3, space="PSUM"))
        psa = pst
        pslin = ctx.enter_context(tc.tile_pool(name="pslin", bufs=2, space="PSUM"))
        pssm = pst
        psout = pslin

        # ---- constants ----
        nc.gpsimd.load_library(library_config.local_scatter)
        ident32 = const.tile([128, 128], F32)
        nc.sync.dma_start(out=ident32[:], in_=id32_d.ap())
        ident16 = const.tile([128, 128], F16)
        nc.sync.dma_start(out=ident16[:], in_=id16_d.ap())
        lt_sb = const.tile([128, N * N], F16)
        lt_ap = lt_d.ap()
        nc.sync.dma_start(
            out=lt_sb[:],
            in_=bass.AP(tensor=lt_ap.tensor, offset=lt_ap.offset,
                        ap=[[0, 128], [1, N * N]]),
        )
        ba_sb = const.tile([128, DT], F32)
        nc.sync.dma_start(out=ba_sb[:], in_=ba_d.ap().rearrange("(t p) -> p t", p=128))
        bb_sb = const.tile([128, DT], F32)
        nc.sync.dma_start(out=bb_sb[:], in_=bb_d.ap().rearrange("(t p) -> p t", p=128))

        # ---- load x (per-batch row tiles) and build X_T [din, r] ----
        xg = x_d.ap()
        xb = []
        for b in range(BPC):
            t = xpool.tile([N, D], F32, tag=f"xb{b}")
            nc.sync.dma_start(out=t[:], in_=xg[b * N:(b + 1) * N, :])
            xb.append(t)

        xt_sb = xt.tile([128, DT, R], F32)
        for b in range(BPC):
            for dk in range(DT):
                pt = pst.tile([128, N], F32, tag="ps")
                nc.tensor.transpose(
                    out=pt[:],
                    in_=xb[b][:, dk * 128:(dk + 1) * 128],
                    identity=ident32[:N, :N],
                )
                nc.vector.tensor_copy(
                    out=xt_sb[:, dk, b * N:(b + 1) * N], in_=pt[:]
                )

        # ---- graph tiles ----
        gg = g_d.ap()
        # row-tiled (for histogram build): [128, N] + [72->80, N]
        gf0 = gpool.tile([128, N], F16, tag="gf0")
        nc.sync.dma_start(out=gf0[:], in_=gg[0:128, :])
        gf1 = gpool.tile([80, N], F16, tag="gf1")
        nc.sync.dma_start(out=gf1[:R - 128, :], in_=gg[128:R, :])
        # batch-tiled (for adjacency): [N, N] per batch
        gb = []
        for b in range(BPC):
            t = gpool.tile([N, N], F16, tag=f"gb{b}")
            nc.sync.dma_start(out=t[:], in_=gg[b * N:(b + 1) * N, :])
            gb.append(t)

        # ---- histogram C[r, l] (fp16), two row tiles ----
        c_tiles = []
        for ti, (gf, rows) in enumerate(((gf0, 128), (gf1, R - 128))):
            meq = cb.tile([128, N, N], F16, tag="meq")
            nc.vector.tensor_tensor(
                out=meq[:rows],
                in0=_bcast3(gf[:rows, :], N, N, "j"),
                in1=_bcast3(gf[:rows, :], N, N, "jp"),
                op=mybir.AluOpType.is_equal,
            )
            cnt32 = cb.tile([128, N], F32, tag="cnt32")
            nc.vector.tensor_reduce(
                out=cnt32[:rows], in_=meq[:rows], axis=mybir.AxisListType.X,
                op=mybir.AluOpType.add,
            )
            mlt = cb.tile([128, N, N], F16, tag="mlt")
            lt_full = lt_sb[:]
            lt3 = bass.AP(tensor=lt_full.tensor, offset=lt_full.offset,
                          ap=[list(lt_full.ap[0]), [N, N], [1, N]])
            nc.vector.tensor_tensor(
                out=mlt[:rows],
                in0=meq[:rows],
                in1=bass.AP(tensor=lt3.tensor, offset=lt3.offset,
                            ap=[[lt3.ap[0][0], rows], [N, N], [1, N]]),
                op=mybir.AluOpType.mult,
            )
            rank32 = cb.tile([128, N], F32, tag="rank32")
            nc.vector.tensor_reduce(
                out=rank32[:rows], in_=mlt[:rows], axis=mybir.AxisListType.X,
                op=mybir.AluOpType.add,
            )
            # first-occurrence mask: rank == 0
            fo = cb.tile([128, N], F16, tag="fo")
            nc.vector.tensor_scalar(
                out=fo[:rows], in0=rank32[:rows], scalar1=0.0, scalar2=None,
                op0=mybir.AluOpType.is_equal,
            )
            # idx = fo * (g + 1) - 1   (-1 where not first occurrence)
            gp1 = cb.tile([128, N], F16, tag="gp1")
            nc.vector.tensor_scalar(
                out=gp1[:rows], in0=gf[:rows, :], scalar1=1.0, scalar2=None,
                op0=mybir.AluOpType.add,
            )
            idxf = cb.tile([128, N], F16, tag="idxf")
            nc.vector.scalar_tensor_tensor(
                out=idxf[:rows], in0=fo[:rows], scalar=-1.0, in1=gp1[:rows],
                op0=mybir.AluOpType.bypass, op1=mybir.AluOpType.mult,
            )
            nc.vector.tensor_scalar(
                out=idxf[:rows], in0=idxf[:rows], scalar1=-1.0, scalar2=None,
                op0=mybir.AluOpType.add,
            )
            nch = 128 if ti == 0 else 80
            idx16 = cb.tile([128, N], I16, tag="idx16")
            cnt16 = cb.tile([128, N], F16, tag="cnt16")
            if rows < nch:
                nc.vector.memset(idx16[:nch, :], -1)
                nc.vector.memset(cnt16[:nch, :], 0.0)
            nc.vector.tensor_copy(out=idx16[:rows], in_=idxf[:rows])
            nc.vector.tensor_copy(out=cnt16[:rows], in_=cnt32[:rows])
            cmat = cpool.tile([nch, L], F16, tag=f"c{ti}")
            nc.gpsimd.local_scatter(
                out_ap=cmat[:],
                data_ap=cnt16[:nch],
                idxs_ap=idx16[:nch],
                channels=nch,
                num_elems=L,
                num_idxs=N,
            )
            c_tiles.append((cmat, rows))

        # ---- A_T / B_T = (x @ Wa.T).T etc, [dout, r] orientation ----
        # weights streamed as column panels WqT[:, dt_i*128:(dt_i+1)*128]
        # reshaped to [128(din), DT, 128(dout)] so each panel is consumed
        # by its 8 matmuls then released.
        at_sb = abp.tile([128, DT, R], F32, tag="at")
        bt_sb = abp.tile([128, DT, R], F32, tag="bt")
        for w_d, bias_col, dst in (
            (wat_d, ba_sb, at_sb), (wbt_d, bb_sb, bt_sb)
        ):
            for dt_i in range(DT):
                panel = wstream.tile([128, DT, 128], F32, tag="wpanel")
                nc.sync.dma_start(
                    out=panel[:],
                    in_=w_d.ap()[:, dt_i * 128:(dt_i + 1) * 128].rearrange(
                        "(t p) c -> p t c", p=128
                    ),
                )
                ps = psa.tile([128, R], F32, tag="ps")
                for dk in range(DT):
                    nc.tensor.matmul(
                        out=ps[:],
                        lhsT=panel[:, dk, :],
                        rhs=xt_sb[:, dk, :],
                        start=(dk == 0),
                        stop=(dk == DT - 1),
                    )
                nc.vector.tensor_scalar(
                    out=dst[:, dt_i, :], in0=ps[:],
                    scalar1=bias_col[:, dt_i:dt_i + 1], scalar2=None,
                    op0=mybir.AluOpType.add,
                )

        # ---- C^T tiles [l, r] f32 via PE transpose ----
        ct_sb = ctp.tile([128, LT_TILES, R], F32)
        for lc in range(LT_TILES):
            cs = min(128, L - lc * 128)
            col = 0
            for (cmat, rows) in c_tiles:
                pt = pst.tile([128, 128], F16, tag="ps")
                nc.tensor.transpose(
                    out=pt[:cs, :cmat.shape[0]],
                    in_=cmat[:, lc * 128:lc * 128 + cs],
                    identity=ident16[:cmat.shape[0], :cmat.shape[0]],
                )
                nc.vector.tensor_copy(
                    out=ct_sb[:cs, lc, col:col + rows], in_=pt[:cs, :rows]
                )
                col += rows

        # ---- LIN[r, d] = x + x@Wc.T + C@bias  (per batch psum) ----
        # stream WcT row tiles / bias label tiles; dk/lc outer so each
        # streamed tile is consumed then released.
        lin_ps = []
        for b in range(BPC):
            lp = pslin.tile([N, D], F32, tag="pslin")
            lin_ps.append(lp)
        for dk in range(DT):
            wt = wstream.tile([128, D], F32, tag="wpanel")
            nc.sync.dma_start(out=wt[:], in_=wct_d.ap()[dk * 128:(dk + 1) * 128, :])
            for b in range(BPC):
                for nch in range(2):
                    sl = slice(nch * 512, (nch + 1) * 512)
                    nc.tensor.matmul(
                        out=lin_ps[b][:, sl],
                        lhsT=xt_sb[:, dk, b * N:(b + 1) * N],
                        rhs=wt[:, sl],
                        start=(dk == 0),
                        stop=False,
                    )
        for lc in range(LT_TILES):
            cs = min(128, L - lc * 128)
            btile = bstream.tile([128, D], F32, tag="btile")
            nc.sync.dma_start(out=btile[:cs], in_=bias_d.ap()[lc * 128:lc * 128 + cs, :])
            for b in range(BPC):
                for nch in range(2):
                    sl = slice(nch * 512, (nch + 1) * 512)
                    nc.tensor.matmul(
                        out=lin_ps[b][:, sl],
                        lhsT=ct_sb[:cs, lc, b * N:(b + 1) * N],
                        rhs=btile[:cs, sl],
                        start=False,
                        stop=(lc == LT_TILES - 1),
                    )
        lin_sb = []
        for b in range(BPC):
            t = linp.tile([N, D], F32, tag=f"lin{b}")
            nc.vector.tensor_add(t[:], lin_ps[b][:], xb[b][:])
            lin_sb.append(t)

        # ---- per-batch attention ----
        for b in range(BPC):
            rsl = slice(b * N, (b + 1) * N)
            # alpha_raw[i, j] = a_i . b_j  -> relu
            psal = pssm.tile([N, N], F32, tag="ps")
            for dk in range(DT):
                nc.tensor.matmul(
                    out=psal[:],
                    lhsT=at_sb[:, dk, rsl],
                    rhs=bt_sb[:, dk, rsl],
                    start=(dk == 0),
                    stop=(dk == DT - 1),
                )
            alpha_sb = small.tile([N, N], F32, tag="alpha")
            nc.vector.tensor_scalar(
                out=alpha_sb[:], in0=psal[:], scalar1=0.0, scalar2=None,
                op0=mybir.AluOpType.max,
            )
            # adjT[j, i] = (g[i, j] != 0)
            psgt = pst.tile([N, N], F16, tag="ps")
            nc.tensor.transpose(out=psgt[:], in_=gb[b][:], identity=ident16[:N, :N])
            adjt_sb = small.tile([N, N], F32, tag="adjt")
            nc.vector.tensor_scalar(
                out=adjt_sb[:], in0=psgt[:], scalar1=0.0, scalar2=None,
                op0=mybir.AluOpType.not_equal,
            )
            # alpha2[i, k] = sum_j adj[i, j] alpha[j, k]
            psal2 = pssm.tile([N, N], F32, tag="ps")
            nc.tensor.matmul(
                out=psal2[:], lhsT=adjt_sb[:], rhs=alpha_sb[:],
                start=True, stop=True,
            )
            al2_sb = small.tile([N, N], F32, tag="al2")
            nc.vector.tensor_copy(out=al2_sb[:], in_=psal2[:])
            # transpose -> [k, i], softmax along free dim (i)
            psal2t = pssm.tile([N, N], F32, tag="ps")
            nc.tensor.transpose(out=psal2t[:], in_=al2_sb[:], identity=ident32[:N, :N])
            negmx = small.tile([N, 1], F32, tag="negmx")
            nc.vector.tensor_reduce(
                out=negmx[:], in_=psal2t[:], axis=mybir.AxisListType.X,
                op=mybir.AluOpType.max, negate=True,
            )
            sm_sb = small.tile([N, N], F32, tag="smexp")
            ssum = small.tile([N, 1], F32, tag="ssum")
            nc.scalar.activation(
                out=sm_sb[:], in_=psal2t[:],
                func=mybir.ActivationFunctionType.Exp,
                bias=negmx[:], scale=1.0, accum_out=ssum[:],
            )
            rsum = small.tile([N, 1], F32, tag="rsum")
            nc.vector.reciprocal(out=rsum[:], in_=ssum[:])
            al3t_sb = small.tile([N, N], F32, tag="al3t")
            nc.vector.tensor_scalar(
                out=al3t_sb[:], in0=sm_sb[:], scalar1=rsum[:], scalar2=None,
                op0=mybir.AluOpType.mult,
            )
            # out[i, d] = sum_k alpha3[i, k] lin[k, d]
            pso = psout.tile([N, D], F32, tag="pslin")
            for nch in range(2):
                sl = slice(nch * 512, (nch + 1) * 512)
                nc.tensor.matmul(
                    out=pso[:, sl], lhsT=al3t_sb[:], rhs=lin_sb[b][:, sl],
                    start=True, stop=True,
                )
            o_sb = outp.tile([N, D], F32, tag="osb")
            nc.vector.tensor_copy(out=o_sb[:], in_=pso[:])
            nc.sync.dma_start(out=out_d.ap()[b * N:(b + 1) * N, :], in_=o_sb[:])


def _prep_inputs(feature, graph, W0, W1, bias, dp_Wa, dp_ba, dp_Wb, dp_bb):
    feature = np.ascontiguousarray(np.asarray(feature, dtype=np.float32))
    graph = np.asarray(graph)
    bias = np.ascontiguousarray(np.asarray(bias, dtype=np.float32))
    wct = np.ascontiguousarray(np.asarray(W0, np.float32).T
                               + np.asarray(W1, np.float32).T)
    wat = np.ascontiguousarray(np.asarray(dp_Wa, np.float32).T)
    wbt = np.ascontiguousarray(np.asarray(dp_Wb, np.float32).T)
    ba = np.ascontiguousarray(np.asarray(dp_ba, np.float32))
    bb = np.ascontiguousarray(np.asarray(dp_bb, np.float32))
    g16 = graph.astype(np.float16)  # labels < 2048: exact in fp16
    j = np.arange(N)
    ltmask = (j[None, :] < j[:, None]).astype(np.float16).reshape(-1)
    ltmask = np.ascontiguousarray(ltmask)

    in_maps = []
    for c in range(NCORES):
        bs = slice(c * BPC, (c + 1) * BPC)
        in_maps.append({
            "x": np.ascontiguousarray(feature[bs].reshape(R, D)),
            "id32": np.eye(128, dtype=np.float32),
            "id16": np.eye(128, dtype=np.float16),
            "g16": np.ascontiguousarray(g16[bs].reshape(R, N)),
            "ltmask": ltmask,
            "wct": wct,
            "wat": wat,
            "wbt": wbt,
            "bias": bias,
            "ba": ba,
            "bb": bb,
        })
    return in_maps


def get_program():
    if "nc" not in _CACHE:
        _CACHE["nc"] = _build_program()
    return _CACHE["nc"]


def kernel(feature, graph, W0, W1, bias, dp_Wa, dp_ba, dp_Wb, dp_bb,
           get_alpha=0, **_ignored):
    from concourse.bass_utils import run_bass_kernel_spmd

    nc = get_program()
    in_maps = _prep_inputs(feature, graph, W0, W1, bias, dp_Wa, dp_ba,
                           dp_Wb, dp_bb)
    res = run_bass_kernel_spmd(nc, in_maps, list(range(NCORES)))
    out = np.concatenate(
        [res.results[c]["out"].reshape(BPC, N, D) for c in range(NCORES)], axis=0
    )
    return out


# revision 16
# speedup vs baseline: 1.3875x; 1.3875x over previous
"""Trainium2 Bass kernel for CorrelatedGraphConv.

Reference computation (per batch b, N=100 rows, D=1024, L=2000 labels):
    adj   = (graph != 0)
    lin   = x + x@W0.T + x@W1.T + sum_j bias[graph[:, j]]
    a     = x@Wa.T + ba ; bvec = x@Wb.T + bb
    alpha = relu(a @ bvec.T)
    alpha = softmax(adj @ alpha, axis=0)   # over rows i
    out   = alpha @ lin

Strategy: data-parallel over batch across 8 cores (2 batches/core).
The label-gather `sum_j bias[g[i,j]]` is computed as `C @ bias` where
C[i, l] = #{j : g[i,j] == l} is built on-chip with gpsimd.local_scatter
(per-partition indexed scatter); duplicate labels within a row are
pre-combined on DVE (count via self-equality matmul-free compare, only
the first occurrence scatters the total count).
"""

import numpy as np

import concourse.bass as bass
import concourse.mybir as mybir
import concourse.tile as tile
from concourse import bacc, library_config

F32 = mybir.dt.float32
F32R = mybir.dt.float32r
F16 = mybir.dt.float16
I16 = mybir.dt.int16

B, N, D, L = 16, 100, 1024, 2000
NCORES = 8
BPC = B // NCORES          # batches per core
R = BPC * N                # rows per core (200)
DT = D // 128              # 8 d-tiles
LT_TILES = (L + 127) // 128  # 16 label tiles (last is 80)

_CACHE = {}


def _bcast3(ap, mid, inner, mode):
    """[P, F] AP -> [P, mid, inner] broadcast view.

    mode 'j':  out[p, a, b] = ap[p, a]   (inner broadcast)
    mode 'jp': out[p, a, b] = ap[p, b]   (middle broadcast)
    """
    (pstep, pcount), (fstep, fcount) = ap.ap[0], ap.ap[1]
    if mode == "j":
        assert fcount == mid
        new = [[pstep, pcount], [fstep, mid], [0, inner]]
    else:
        assert fcount == inner
        new = [[pstep, pcount], [0, mid], [fstep, inner]]
    return bass.AP(tensor=ap.tensor, offset=ap.offset, ap=new)


def _pbcast(ap, p):
    """[1, ...] AP -> [p, ...] partition-broadcast view."""
    new = [[0, p]] + [list(d) for d in ap.ap[1:]]
    return bass.AP(tensor=ap.tensor, offset=ap.offset, ap=new)


def _build_program():
    nc = bacc.Bacc("TRN2", target_bir_lowering=False, debug=False,
                   num_devices=NCORES)

    x_d = nc.declare_dram_parameter("x", [R, D], F32, isOutput=False)
    id32_d = nc.declare_dram_parameter("id32", [128, 128], F32, isOutput=False)
    id16_d = nc.declare_dram_parameter("id16", [128, 128], F16, isOutput=False)
    g_d = nc.declare_dram_parameter("g16", [R, N], F16, isOutput=False)
    lt_d = nc.declare_dram_parameter("ltmask", [N * N], F16, isOutput=False)
    wct_d = nc.declare_dram_parameter("wct", [D, D], F32R, isOutput=False)
    wat_d = nc.declare_dram_parameter("wat", [D, D], F32R, isOutput=False)
    wbt_d = nc.declare_dram_parameter("wbt", [D, D], F32R, isOutput=False)
    bias_d = nc.declare_dram_parameter("bias", [L, D], F32R, isOutput=False)
    ba_d = nc.declare_dram_parameter("ba", [D], F32, isOutput=False)
    bb_d = nc.declare_dram_parameter("bb", [D], F32, isOutput=False)
    out_d = nc.declare_dram_parameter("out", [R, D], F32, isOutput=True)

    with tile.TileContext(nc) as tc:
        _emit(tc, x_d, id32_d, id16_d, g_d, lt_d, wct_d, wat_d, wbt_d,
              bias_d, ba_d, bb_d, out_d)
    nc.compile()
    return nc


def _emit(tc, x_d, id32_d, id16_d, g_d, lt_d, wct_d, wat_d, wbt_d, bias_d,
          ba_d, bb_d, out_d):
    nc = tc.nc
    import contextlib

    ctx = contextlib.ExitStack()
    with ctx:
        const = ctx.enter_context(tc.tile_pool(name="const", bufs=1))
        gpool = ctx.enter_context(tc.tile_pool(name="gtiles", bufs=1))
        xpool = ctx.enter_context(tc.tile_pool(name="xtiles", bufs=1))
        xt = ctx.enter_context(tc.tile_pool(name="xt", bufs=1))
        abp = ctx.enter_context(tc.tile_pool(name="abt", bufs=1))
        linp = ctx.enter_context(tc.tile_pool(name="lin", bufs=1))
        cb = ctx.enter_context(tc.tile_pool(name="cbuild", bufs=1))
        cpool = ctx.enter_context(tc.tile_pool(name="cmat", bufs=1))
        ctp = ctx.enter_context(tc.tile_pool(name="ctmat", bufs=1))
        wstream = ctx.enter_context(tc.tile_pool(name="wstream", bufs=4))
        bstream = ctx.enter_context(tc.tile_pool(name="bstream", bufs=4))
        small = ctx.enter_context(tc.tile_pool(name="small", bufs=2))
        outp = ctx.enter_context(tc.tile_pool(name="outs", bufs=2))
        pst = ctx.enter_context(tc.tile_pool(name="pst", bufs=3, space="PSUM"))
        psa = pst
        pslin = ctx.enter_context(tc.tile_pool(name="pslin", bufs=2, space="PSUM"))
        pssm = pst
        psout = pslin

        # ---- constants ----
        nc.gpsimd.load_library(library_config.local_scatter)
        ident32 = const.tile([128, 128], F32)
        nc.sync.dma_start(out=ident32[:], in_=id32_d.ap())
        ident16 = const.tile([128, 128], F16)
        nc.sync.dma_start(out=ident16[:], in_=id16_d.ap())
        lt_sb = const.tile([128, N * N], F16)
        lt_ap = lt_d.ap()
        nc.sync.dma_start(
            out=lt_sb[:],
            in_=bass.AP(tensor=lt_ap.tensor, offset=lt_ap.offset,
                        ap=[[0, 128], [1, N * N]]),
        )
        ba_sb = const.tile([128, DT], F32)
        nc.sync.dma_start(out=ba_sb[:], in_=ba_d.ap().rearrange("(t p) -> p t", p=128))
        bb_sb = const.tile([128, DT], F32)
        nc.sync.dma_start(out=bb_sb[:], in_=bb_d.ap().rearrange("(t p) -> p t", p=128))

        # ---- load x (per-batch row tiles) and build X_T [din, r] ----
        xg = x_d.ap()
        xb = []
        for b in range(BPC):
            t = xpool.tile([N, D], F32, tag=f"xb{b}")
            nc.sync.dma_start(out=t[:], in_=xg[b * N:(b + 1) * N, :])
            xb.append(t)

        RP = 256  # padded row dim for full-rate f32r matmul
        xt_sb = xt.tile([128, DT, RP], F32R)
        nc.vector.memset(xt_sb[:, :, R:RP].bitcast(F32), 0.0)
        for b in range(BPC):
            for dk in range(DT):
                pt = pst.tile([128, N], F32, tag="ps")
                nc.tensor.transpose(
                    out=pt[:],
                    in_=xb[b][:, dk * 128:(dk + 1) * 128],
                    identity=ident32[:N, :N],
                )
                nc.vector.tensor_copy(
                    out=xt_sb[:, dk, b * N:(b + 1) * N], in_=pt[:]
                )

        # ---- graph tiles ----
        gg = g_d.ap()
        # row-tiled (for histogram build): [128, N] + [72->80, N]
        gf0 = gpool.tile([128, N], F16, tag="gf0")
        nc.sync.dma_start(out=gf0[:], in_=gg[0:128, :])
        gf1 = gpool.tile([80, N], F16, tag="gf1")
        nc.sync.dma_start(out=gf1[:R - 128, :], in_=gg[128:R, :])
        # batch-tiled (for adjacency): [N, N] per batch
        gb = []
        for b in range(BPC):
            t = gpool.tile([N, N], F16, tag=f"gb{b}")
            nc.sync.dma_start(out=t[:], in_=gg[b * N:(b + 1) * N, :])
            gb.append(t)

        # ---- histogram C[r, l] (fp16), two row tiles ----
        c_tiles = []
        for ti, (gf, rows) in enumerate(((gf0, 128), (gf1, R - 128))):
            meq = cb.tile([128, N, N], F16, tag="meq")
            nc.vector.tensor_tensor(
                out=meq[:rows],
                in0=_bcast3(gf[:rows, :], N, N, "j"),
                in1=_bcast3(gf[:rows, :], N, N, "jp"),
                op=mybir.AluOpType.is_equal,
            )
            cnt32 = cb.tile([128, N], F32, tag="cnt32")
            nc.vector.tensor_reduce(
                out=cnt32[:rows], in_=meq[:rows], axis=mybir.AxisListType.X,
                op=mybir.AluOpType.add,
            )
            lt_full = lt_sb[:]
            nc.vector.tensor_tensor(
                out=meq[:rows],
                in0=meq[:rows],
                in1=bass.AP(tensor=lt_full.tensor, offset=lt_full.offset,
                            ap=[[lt_full.ap[0][0], rows], [N, N], [1, N]]),
                op=mybir.AluOpType.mult,
            )
            rank32 = cb.tile([128, N], F32, tag="rank32")
            nc.vector.tensor_reduce(
                out=rank32[:rows], in_=meq[:rows], axis=mybir.AxisListType.X,
                op=mybir.AluOpType.add,
            )
            # first-occurrence mask: rank == 0
            fo = cb.tile([128, N], F16, tag="fo")
            nc.vector.tensor_scalar(
                out=fo[:rows], in0=rank32[:rows], scalar1=0.0, scalar2=None,
                op0=mybir.AluOpType.is_equal,
            )
            # idx = fo * (g + 1) - 1   (-1 where not first occurrence)
            gp1 = cb.tile([128, N], F16, tag="gp1")
            nc.vector.tensor_scalar(
                out=gp1[:rows], in0=gf[:rows, :], scalar1=1.0, scalar2=None,
                op0=mybir.AluOpType.add,
            )
            idxf = cb.tile([128, N], F16, tag="idxf")
            nc.vector.scalar_tensor_tensor(
                out=idxf[:rows], in0=fo[:rows], scalar=-1.0, in1=gp1[:rows],
                op0=mybir.AluOpType.bypass, op1=mybir.AluOpType.mult,
            )
            nc.vector.tensor_scalar(
                out=idxf[:rows], in0=idxf[:rows], scalar1=-1.0, scalar2=None,
                op0=mybir.AluOpType.add,
            )
            nch = 128 if ti == 0 else 80
            idx16 = cb.tile([128, N], I16, tag="idx16")
            cnt16 = cb.tile([128, N], F16, tag="cnt16")
            if rows < nch:
                nc.vector.memset(idx16[:nch, :], -1)
                nc.vector.memset(cnt16[:nch, :], 0.0)
            nc.vector.tensor_copy(out=idx16[:rows], in_=idxf[:rows])
            nc.vector.tensor_copy(out=cnt16[:rows], in_=cnt32[:rows])
            cmat = cpool.tile([nch, L], F16, tag=f"c{ti}")
            nc.gpsimd.local_scatter(
                out_ap=cmat[:],
                data_ap=cnt16[:nch],
                idxs_ap=idx16[:nch],
                channels=nch,
                num_elems=L,
                num_idxs=N,
            )
            c_tiles.append((cmat, rows))

        # ---- A_T / B_T = (x @ Wa.T).T etc, [dout, r] orientation ----
        # weights streamed as column panels WqT[:, dt_i*128:(dt_i+1)*128]
        # reshaped to [128(din), DT, 128(dout)] so each panel is consumed
        # by its 8 matmuls then released.
        at_sb = abp.tile([128, DT, R], F32R, tag="at")
        bt_sb = abp.tile([128, DT, R], F32R, tag="bt")
        for w_d, bias_col, dst in (
            (wat_d, ba_sb, at_sb), (wbt_d, bb_sb, bt_sb)
        ):
            for dt_i in range(DT):
                panel = wstream.tile([128, DT, 128], F32R, tag="wpanel")
                nc.sync.dma_start(
                    out=panel[:],
                    in_=w_d.ap()[:, dt_i * 128:(dt_i + 1) * 128].rearrange(
                        "(t p) c -> p t c", p=128
                    ),
                )
                ps = psa.tile([128, RP], F32, tag="ps")
                for dk in range(DT):
                    nc.tensor.matmul(
                        out=ps[:],
                        lhsT=panel[:, dk, :],
                        rhs=xt_sb[:, dk, :],
                        start=(dk == 0),
                        stop=(dk == DT - 1),
                    )
                nc.vector.tensor_scalar(
                    out=dst[:, dt_i, :], in0=ps[:, 0:R],
                    scalar1=bias_col[:, dt_i:dt_i + 1], scalar2=None,
                    op0=mybir.AluOpType.add,
                )

        # ---- C^T tiles [l, r] f32 via PE transpose ----
        ct_sb = ctp.tile([128, LT_TILES, R], F32R)
        for lc in range(LT_TILES):
            cs = min(128, L - lc * 128)
            col = 0
            for (cmat, rows) in c_tiles:
                pt = pst.tile([128, 128], F16, tag="ps")
                nc.tensor.transpose(
                    out=pt[:cs, :cmat.shape[0]],
                    in_=cmat[:, lc * 128:lc * 128 + cs],
                    identity=ident16[:cmat.shape[0], :cmat.shape[0]],
                )
                nc.vector.tensor_copy(
                    out=ct_sb[:cs, lc, col:col + rows], in_=pt[:cs, :rows]
                )
                col += rows

        # ---- LIN[r, d] = x + x@Wc.T + C@bias  (per batch psum) ----
        # stream WcT row tiles / bias label tiles; dk/lc outer so each
        # streamed tile is consumed then released.
        lin_ps = []
        for b in range(BPC):
            lp = pslin.tile([N, D], F32, tag="pslin")
            lin_ps.append(lp)
        for dk in range(DT):
            wt = wstream.tile([128, D], F32R, tag="wpanel")
            nc.sync.dma_start(out=wt[:], in_=wct_d.ap()[dk * 128:(dk + 1) * 128, :])
            for b in range(BPC):
                for nch in range(2):
                    sl = slice(nch * 512, (nch + 1) * 512)
                    nc.tensor.matmul(
                        out=lin_ps[b][:, sl],
                        lhsT=xt_sb[:, dk, b * N:(b + 1) * N],
                        rhs=wt[:, sl],
                        start=(dk == 0),
                        stop=False,
                    )
        for lc in range(LT_TILES):
            cs = min(128, L - lc * 128)
            btile = bstream.tile([128, D], F32R, tag="btile")
            nc.sync.dma_start(out=btile[:cs], in_=bias_d.ap()[lc * 128:lc * 128 + cs, :])
            for b in range(BPC):
                for nch in range(2):
                    sl = slice(nch * 512, (nch + 1) * 512)
                    nc.tensor.matmul(
                        out=lin_ps[b][:, sl],
                        lhsT=ct_sb[:cs, lc, b * N:(b + 1) * N],
                        rhs=btile[:cs, sl],
                        start=False,
                        stop=(lc == LT_TILES - 1),
                    )
        lin_sb = []
        for b in range(BPC):
            t = linp.tile([N, D], F32R, tag=f"lin{b}")
            nc.vector.tensor_add(t[:], lin_ps[b][:], xb[b][:])
            lin_sb.append(t)

        # ---- per-batch attention ----
        for b in range(BPC):
            rsl = slice(b * N, (b + 1) * N)
            # alpha_raw[i, j] = a_i . b_j  -> relu
            psal = pssm.tile([N, N], F32, tag="ps")
            for dk in range(DT):
                nc.tensor.matmul(
                    out=psal[:],
                    lhsT=at_sb[:, dk, rsl],
                    rhs=bt_sb[:, dk, rsl],
                    start=(dk == 0),
                    stop=(dk == DT - 1),
                )
            alpha_sb = small.tile([N, N], F32R, tag="alpha")
            nc.vector.tensor_scalar(
                out=alpha_sb[:], in0=psal[:], scalar1=0.0, scalar2=None,
                op0=mybir.AluOpType.max,
            )
            # adjT[j, i] = (g[i, j] != 0)
            psgt = pst.tile([N, N], F16, tag="ps")
            nc.tensor.transpose(out=psgt[:], in_=gb[b][:], identity=ident16[:N, :N])
            adjt_sb = small.tile([N, N], F32R, tag="adjt")
            nc.vector.tensor_scalar(
                out=adjt_sb[:], in0=psgt[:], scalar1=0.0, scalar2=None,
                op0=mybir.AluOpType.not_equal,
            )
            # alpha2[i, k] = sum_j adj[i, j] alpha[j, k]
            psal2 = pssm.tile([N, N], F32, tag="ps")
            nc.tensor.matmul(
                out=psal2[:], lhsT=adjt_sb[:], rhs=alpha_sb[:],
                start=True, stop=True,
            )
            al2_sb = small.tile([N, N], F32, tag="al2")
            nc.vector.tensor_copy(out=al2_sb[:], in_=psal2[:])
            # transpose -> [k, i], softmax along free dim (i)
            psal2t = pssm.tile([N, N], F32, tag="ps")
            nc.tensor.transpose(out=psal2t[:], in_=al2_sb[:], identity=ident32[:N, :N])
            negmx = small.tile([N, 1], F32, tag="negmx")
            nc.vector.tensor_reduce(
                out=negmx[:], in_=psal2t[:], axis=mybir.AxisListType.X,
                op=mybir.AluOpType.max, negate=True,
            )
            sm_sb = small.tile([N, N], F32, tag="smexp")
            ssum = small.tile([N, 1], F32, tag="ssum")
            nc.scalar.activation(
                out=sm_sb[:], in_=psal2t[:],
                func=mybir.ActivationFunctionType.Exp,
                bias=negmx[:], scale=1.0, accum_out=ssum[:],
            )
            rsum = small.tile([N, 1], F32, tag="rsum")
            nc.vector.reciprocal(out=rsum[:], in_=ssum[:])
            al3t_sb = small.tile([N, N], F32R, tag="al3t")
            nc.vector.tensor_scalar(
                out=al3t_sb[:], in0=sm_sb[:], scalar1=rsum[:], scalar2=None,
                op0=mybir.AluOpType.mult,
            )
            # out[i, d] = sum_k alpha3[i, k] lin[k, d]
            pso = psout.tile([N, D], F32, tag="pslin")
            for nch in range(2):
                sl = slice(nch * 512, (nch + 1) * 512)
                nc.tensor.matmul(
                    out=pso[:, sl], lhsT=al3t_sb[:], rhs=lin_sb[b][:, sl],
                    start=True, stop=True,
                )
            o_sb = outp.tile([N, D], F32, tag="osb")
            nc.vector.tensor_copy(out=o_sb[:], in_=pso[:])
            nc.sync.dma_start(out=out_d.ap()[b * N:(b + 1) * N, :], in_=o_sb[:])


def _prep_inputs(feature, graph, W0, W1, bias, dp_Wa, dp_ba, dp_Wb, dp_bb):
    feature = np.ascontiguousarray(np.asarray(feature, dtype=np.float32))
    graph = np.asarray(graph)
    bias = np.ascontiguousarray(np.asarray(bias, dtype=np.float32))
    wct = np.ascontiguousarray(np.asarray(W0, np.float32).T
                               + np.asarray(W1, np.float32).T)
    wat = np.ascontiguousarray(np.asarray(dp_Wa, np.float32).T)
    wbt = np.ascontiguousarray(np.asarray(dp_Wb, np.float32).T)
    ba = np.ascontiguousarray(np.asarray(dp_ba, np.float32))
    bb = np.ascontiguousarray(np.asarray(dp_bb, np.float32))
    g16 = graph.astype(np.float16)  # labels < 2048: exact in fp16
    j = np.arange(N)
    ltmask = (j[None, :] < j[:, None]).astype(np.float16).reshape(-1)
    ltmask = np.ascontiguousarray(ltmask)

    in_maps = []
    for c in range(NCORES):
        bs = slice(c * BPC, (c + 1) * BPC)
        in_maps.append({
            "x": np.ascontiguousarray(feature[bs].reshape(R, D)),
            "id32": np.eye(128, dtype=np.float32),
            "id16": np.eye(128, dtype=np.float16),
            "g16": np.ascontiguousarray(g16[bs].reshape(R, N)),
            "ltmask": ltmask,
            "wct": wct,
            "wat": wat,
            "wbt": wbt,
            "bias": bias,
            "ba": ba,
            "bb": bb,
        })
    return in_maps


def get_program():
    if "nc" not in _CACHE:
        _CACHE["nc"] = _build_program()
    return _CACHE["nc"]


def kernel(feature, graph, W0, W1, bias, dp_Wa, dp_ba, dp_Wb, dp_bb,
           get_alpha=0, **_ignored):
    from concourse.bass_utils import run_bass_kernel_spmd

    nc = get_program()
    in_maps = _prep_inputs(feature, graph, W0, W1, bias, dp_Wa, dp_ba,
                           dp_Wb, dp_bb)
    res = run_bass_kernel_spmd(nc, in_maps, list(range(NCORES)))
    out = np.concatenate(
        [res.results[c]["out"].reshape(BPC, N, D) for c in range(NCORES)], axis=0
    )
    return out


# revision 21
# speedup vs baseline: 1.4613x; 1.0532x over previous
"""Trainium2 Bass kernel for CorrelatedGraphConv.

Reference computation (per batch b, N=100 rows, D=1024, L=2000 labels):
    adj   = (graph != 0)
    lin   = x + x@W0.T + x@W1.T + sum_j bias[graph[:, j]]
    a     = x@Wa.T + ba ; bvec = x@Wb.T + bb
    alpha = relu(a @ bvec.T)
    alpha = softmax(adj @ alpha, axis=0)   # over rows i
    out   = alpha @ lin

Strategy: data-parallel over batch across 8 cores (2 batches/core).
The label-gather `sum_j bias[g[i,j]]` is computed as `C @ bias` where
C[i, l] = #{j : g[i,j] == l} is built on-chip with gpsimd.local_scatter
(per-partition indexed scatter); duplicate labels within a row are
pre-combined on DVE (count via self-equality matmul-free compare, only
the first occurrence scatters the total count).
"""

import numpy as np

import concourse.bass as bass
import concourse.mybir as mybir
import concourse.tile as tile
from concourse import bacc, library_config

F32 = mybir.dt.float32
F32R = mybir.dt.float32r
F16 = mybir.dt.float16
I16 = mybir.dt.int16

B, N, D, L = 16, 100, 1024, 2000
NCORES = 8
BPC = B // NCORES          # batches per core
R = BPC * N                # rows per core (200)
DT = D // 128              # 8 d-tiles
LT_TILES = (L + 127) // 128  # 16 label tiles (last is 80)
LPAD = 2048                  # padded label rows in scatter tables
ESC = 256                    # scatter element width (fp16) = 512B rows
NCALL = 5                    # scatter calls (one table each)
TPC = R * N // NCALL         # tokens per call (10000)
SCHUNK = (TPC + 127) // 128  # source chunks per call (79)

_CACHE = {}


def _bcast3(ap, mid, inner, mode):
    """[P, F] AP -> [P, mid, inner] broadcast view.

    mode 'j':  out[p, a, b] = ap[p, a]   (inner broadcast)
    mode 'jp': out[p, a, b] = ap[p, b]   (middle broadcast)
    """
    (pstep, pcount), (fstep, fcount) = ap.ap[0], ap.ap[1]
    if mode == "j":
        assert fcount == mid
        new = [[pstep, pcount], [fstep, mid], [0, inner]]
    else:
        assert fcount == inner
        new = [[pstep, pcount], [0, mid], [fstep, inner]]
    return bass.AP(tensor=ap.tensor, offset=ap.offset, ap=new)


def _pbcast(ap, p):
    """[1, ...] AP -> [p, ...] partition-broadcast view."""
    new = [[0, p]] + [list(d) for d in ap.ap[1:]]
    return bass.AP(tensor=ap.tensor, offset=ap.offset, ap=new)


def _build_program():
    nc = bacc.Bacc("TRN2", target_bir_lowering=False, debug=False,
                   num_devices=NCORES)

    x_d = nc.declare_dram_parameter("x", [R, D], F32, isOutput=False)
    id32_d = nc.declare_dram_parameter("id32", [128, 128], F32, isOutput=False)
    id16_d = nc.declare_dram_parameter("id16", [128, 128], F16, isOutput=False)
    g_d = nc.declare_dram_parameter("g16", [R, N], F16, isOutput=False)
    lt_d = nc.declare_dram_parameter("ltmask", [N * N], F16, isOutput=False)
    wct_d = nc.declare_dram_parameter("wct", [D, D], F32R, isOutput=False)
    wat_d = nc.declare_dram_parameter("wat", [D, D], F32R, isOutput=False)
    wbt_d = nc.declare_dram_parameter("wbt", [D, D], F32R, isOutput=False)
    bias_d = nc.declare_dram_parameter("bias", [L, D], F32R, isOutput=False)
    ba_d = nc.declare_dram_parameter("ba", [D], F32, isOutput=False)
    bb_d = nc.declare_dram_parameter("bb", [D], F32, isOutput=False)
    out_d = nc.declare_dram_parameter("out", [R, D], F32, isOutput=True)

    with tile.TileContext(nc) as tc:
        _emit(tc, x_d, id32_d, id16_d, g_d, lt_d, wct_d, wat_d, wbt_d,
              bias_d, ba_d, bb_d, out_d)
    nc.compile()
    return nc


def _emit(tc, x_d, id32_d, id16_d, g_d, lt_d, wct_d, wat_d, wbt_d, bias_d,
          ba_d, bb_d, out_d):
    nc = tc.nc
    import contextlib

    ctx = contextlib.ExitStack()
    with ctx:
        const = ctx.enter_context(tc.tile_pool(name="const", bufs=1))
        gpool = ctx.enter_context(tc.tile_pool(name="gtiles", bufs=1))
        xpool = ctx.enter_context(tc.tile_pool(name="xtiles", bufs=1))
        xt = ctx.enter_context(tc.tile_pool(name="xt", bufs=1))
        abp = ctx.enter_context(tc.tile_pool(name="abt", bufs=1))
        linp = ctx.enter_context(tc.tile_pool(name="lin", bufs=1))
        cb = ctx.enter_context(tc.tile_pool(name="cbuild", bufs=1))
        cpool = ctx.enter_context(tc.tile_pool(name="cmat", bufs=1))
        ctp = ctx.enter_context(tc.tile_pool(name="ctmat", bufs=1))
        wstream = ctx.enter_context(tc.tile_pool(name="wstream", bufs=4))
        bstream = ctx.enter_context(tc.tile_pool(name="bstream", bufs=4))
        small = ctx.enter_context(tc.tile_pool(name="small", bufs=2))
        outp = ctx.enter_context(tc.tile_pool(name="outs", bufs=2))
        pst = ctx.enter_context(tc.tile_pool(name="pst", bufs=3, space="PSUM"))
        psa = pst
        pslin = ctx.enter_context(tc.tile_pool(name="pslin", bufs=2, space="PSUM"))
        pssm = pst
        psout = pslin

        # ---- constants ----
        nc.gpsimd.load_library(library_config.local_scatter)
        ident32 = const.tile([128, 128], F32)
        nc.sync.dma_start(out=ident32[:], in_=id32_d.ap())
        ident16 = const.tile([128, 128], F16)
        nc.sync.dma_start(out=ident16[:], in_=id16_d.ap())
        lt_sb = const.tile([128, N * N], F16)
        lt_ap = lt_d.ap()
        nc.sync.dma_start(
            out=lt_sb[:],
            in_=bass.AP(tensor=lt_ap.tensor, offset=lt_ap.offset,
                        ap=[[0, 128], [1, N * N]]),
        )
        ba_sb = const.tile([128, DT], F32)
        nc.sync.dma_start(out=ba_sb[:], in_=ba_d.ap().rearrange("(t p) -> p t", p=128))
        bb_sb = const.tile([128, DT], F32)
        nc.sync.dma_start(out=bb_sb[:], in_=bb_d.ap().rearrange("(t p) -> p t", p=128))

        # ---- load x (per-batch row tiles) and build X_T [din, r] ----
        xg = x_d.ap()
        xb = []
        for b in range(BPC):
            t = xpool.tile([N, D], F32, tag=f"xb{b}")
            nc.sync.dma_start(out=t[:], in_=xg[b * N:(b + 1) * N, :])
            xb.append(t)

        RP = 256  # padded row dim for full-rate f32r matmul
        xt_sb = xt.tile([128, DT, RP], F32R)
        nc.vector.memset(xt_sb[:, :, R:RP].bitcast(F32), 0.0)
        for b in range(BPC):
            for dk in range(DT):
                pt = pst.tile([128, N], F32, tag="ps")
                nc.tensor.transpose(
                    out=pt[:],
                    in_=xb[b][:, dk * 128:(dk + 1) * 128],
                    identity=ident32[:N, :N],
                )
                nc.vector.tensor_copy(
                    out=xt_sb[:, dk, b * N:(b + 1) * N], in_=pt[:]
                )

        # ---- graph tiles ----
        gg = g_d.ap()
        gf0 = gpool.tile([128, N], F16, tag="gf0")
        nc.sync.dma_start(out=gf0[:], in_=gg[0:128, :])
        gf1 = gpool.tile([80, N], F16, tag="gf1")
        nc.sync.dma_start(out=gf1[:R - 128, :], in_=gg[128:R, :])
        gb = []
        for b in range(BPC):
            t = gpool.tile([N, N], F16, tag=f"gb{b}")
            nc.sync.dma_start(out=t[:], in_=gg[b * N:(b + 1) * N, :])
            gb.append(t)

        # ---- histogram C[r, l] (fp16) via local_scatter, two row tiles ----
        c_tiles = []
        for ti, (gf, rows) in enumerate(((gf0, 128), (gf1, R - 128))):
            meq = cb.tile([128, N, N], F16, tag="meq")
            nc.vector.tensor_tensor(
                out=meq[:rows],
                in0=_bcast3(gf[:rows, :], N, N, "j"),
                in1=_bcast3(gf[:rows, :], N, N, "jp"),
                op=mybir.AluOpType.is_equal,
            )
            cnt32 = cb.tile([128, N], F32, tag="cnt32")
            nc.vector.tensor_reduce(
                out=cnt32[:rows], in_=meq[:rows], axis=mybir.AxisListType.X,
                op=mybir.AluOpType.add,
            )
            lt_full = lt_sb[:]
            nc.vector.tensor_tensor(
                out=meq[:rows],
                in0=meq[:rows],
                in1=bass.AP(tensor=lt_full.tensor, offset=lt_full.offset,
                            ap=[[lt_full.ap[0][0], rows], [N, N], [1, N]]),
                op=mybir.AluOpType.mult,
            )
            rank32 = cb.tile([128, N], F32, tag="rank32")
            nc.vector.tensor_reduce(
                out=rank32[:rows], in_=meq[:rows], axis=mybir.AxisListType.X,
                op=mybir.AluOpType.add,
            )
            fo = cb.tile([128, N], F16, tag="fo")
            nc.vector.tensor_scalar(
                out=fo[:rows], in0=rank32[:rows], scalar1=0.0, scalar2=None,
                op0=mybir.AluOpType.is_equal,
            )
            gp1 = cb.tile([128, N], F16, tag="gp1")
            nc.vector.tensor_scalar(
                out=gp1[:rows], in0=gf[:rows, :], scalar1=1.0, scalar2=None,
                op0=mybir.AluOpType.add,
            )
            idxf = cb.tile([128, N], F16, tag="idxf")
            nc.vector.scalar_tensor_tensor(
                out=idxf[:rows], in0=fo[:rows], scalar=-1.0, in1=gp1[:rows],
                op0=mybir.AluOpType.bypass, op1=mybir.AluOpType.mult,
            )
            nc.vector.tensor_scalar(
                out=idxf[:rows], in0=idxf[:rows], scalar1=-1.0, scalar2=None,
                op0=mybir.AluOpType.add,
            )
            nch = 128 if ti == 0 else 80
            idx16 = cb.tile([128, N], I16, tag="idx16")
            cnt16 = cb.tile([128, N], F16, tag="cnt16")
            if rows < nch:
                nc.vector.memset(idx16[:nch, :], -1)
                nc.vector.memset(cnt16[:nch, :], 0.0)
            nc.vector.tensor_copy(out=idx16[:rows], in_=idxf[:rows])
            nc.vector.tensor_copy(out=cnt16[:rows], in_=cnt32[:rows])
            cmat = cpool.tile([nch, L], F16, tag=f"c{ti}")
            nc.gpsimd.local_scatter(
                out_ap=cmat[:],
                data_ap=cnt16[:nch],
                idxs_ap=idx16[:nch],
                channels=nch,
                num_elems=L,
                num_idxs=N,
            )
            c_tiles.append((cmat, rows))

        # ---- A_T / B_T = (x @ Wa.T).T etc, [dout, r] orientation ----
        # weights streamed as column panels WqT[:, dt_i*128:(dt_i+1)*128]
        # reshaped to [128(din), DT, 128(dout)] so each panel is consumed
        # by its 8 matmuls then released.
        at_sb = abp.tile([128, DT, R], F32R, tag="at")
        bt_sb = abp.tile([128, DT, R], F32R, tag="bt")
        for w_d, bias_col, dst in (
            (wat_d, ba_sb, at_sb), (wbt_d, bb_sb, bt_sb)
        ):
            for dt_i in range(DT):
                panel = wstream.tile([128, DT, 128], F32R, tag="wpanel")
                nc.sync.dma_start(
                    out=panel[:],
                    in_=w_d.ap()[:, dt_i * 128:(dt_i + 1) * 128].rearrange(
                        "(t p) c -> p t c", p=128
                    ),
                )
                ps = psa.tile([128, RP], F32, tag="ps")
                for dk in range(DT):
                    nc.tensor.matmul(
                        out=ps[:],
                        lhsT=panel[:, dk, :],
                        rhs=xt_sb[:, dk, :],
                        start=(dk == 0),
                        stop=(dk == DT - 1),
                    )
                nc.vector.tensor_scalar(
                    out=dst[:, dt_i, :], in0=ps[:, 0:R],
                    scalar1=bias_col[:, dt_i:dt_i + 1], scalar2=None,
                    op0=mybir.AluOpType.add,
                )

        # ---- C^T tiles [l, r] via PE transpose ----
        ct_sb = ctp.tile([128, LT_TILES, R], F32R)
        for lc in range(LT_TILES):
            cs = min(128, L - lc * 128)
            col = 0
            for (cmat, rows) in c_tiles:
                pt = pst.tile([128, 128], F16, tag="ps")
                nc.tensor.transpose(
                    out=pt[:cs, :cmat.shape[0]],
                    in_=cmat[:, lc * 128:lc * 128 + cs],
                    identity=ident16[:cmat.shape[0], :cmat.shape[0]],
                )
                nc.vector.tensor_copy(
                    out=ct_sb[:cs, lc, col:col + rows], in_=pt[:cs, :rows]
                )
                col += rows

        # ---- LIN[r, d] = x + x@Wc.T + C@bias  (per batch psum) ----
        # stream WcT row tiles / bias label tiles; dk/lc outer so each
        # streamed tile is consumed then released.
        lin_ps = []
        for b in range(BPC):
            lp = pslin.tile([N, D], F32, tag="pslin")
            lin_ps.append(lp)
        for dk in range(DT):
            wt = wstream.tile([128, D], F32R, tag="wpanel")
            nc.sync.dma_start(out=wt[:], in_=wct_d.ap()[dk * 128:(dk + 1) * 128, :])
            for b in range(BPC):
                for nch in range(2):
                    sl = slice(nch * 512, (nch + 1) * 512)
                    nc.tensor.matmul(
                        out=lin_ps[b][:, sl],
                        lhsT=xt_sb[:, dk, b * N:(b + 1) * N],
                        rhs=wt[:, sl],
                        start=(dk == 0),
                        stop=False,
                    )
        for lc in range(LT_TILES):
            cs = min(128, L - lc * 128)
            btile = bstream.tile([128, D], F32R, tag="btile")
            nc.sync.dma_start(out=btile[:cs], in_=bias_d.ap()[lc * 128:lc * 128 + cs, :])
            for b in range(BPC):
                for nch in range(2):
                    sl = slice(nch * 512, (nch + 1) * 512)
                    nc.tensor.matmul(
                        out=lin_ps[b][:, sl],
                        lhsT=ct_sb[:cs, lc, b * N:(b + 1) * N],
                        rhs=btile[:cs, sl],
                        start=False,
                        stop=(lc == LT_TILES - 1),
                    )
        lin_sb = []
        for b in range(BPC):
            t = linp.tile([N, D], F32R, tag=f"lin{b}")
            nc.vector.tensor_add(t[:], lin_ps[b][:], xb[b][:])
            lin_sb.append(t)

        # ---- per-batch attention ----
        for b in range(BPC):
            rsl = slice(b * N, (b + 1) * N)
            # alpha_raw[i, j] = a_i . b_j  -> relu
            psal = pssm.tile([N, N], F32, tag="ps")
            for dk in range(DT):
                nc.tensor.matmul(
                    out=psal[:],
                    lhsT=at_sb[:, dk, rsl],
                    rhs=bt_sb[:, dk, rsl],
                    start=(dk == 0),
                    stop=(dk == DT - 1),
                )
            alpha_sb = small.tile([N, N], F32R, tag="alpha")
            nc.vector.tensor_scalar(
                out=alpha_sb[:], in0=psal[:], scalar1=0.0, scalar2=None,
                op0=mybir.AluOpType.max,
            )
            # adjT[j, i] = (g[i, j] != 0)
            psgt = pst.tile([N, N], F16, tag="ps")
            nc.tensor.transpose(out=psgt[:], in_=gb[b][:], identity=ident16[:N, :N])
            adjt_sb = small.tile([N, N], F32R, tag="adjt")
            nc.vector.tensor_scalar(
                out=adjt_sb[:], in0=psgt[:], scalar1=0.0, scalar2=None,
                op0=mybir.AluOpType.not_equal,
            )
            # alpha2[i, k] = sum_j adj[i, j] alpha[j, k]
            psal2 = pssm.tile([N, N], F32, tag="ps")
            nc.tensor.matmul(
                out=psal2[:], lhsT=adjt_sb[:], rhs=alpha_sb[:],
                start=True, stop=True,
            )
            al2_sb = small.tile([N, N], F32, tag="al2")
            nc.vector.tensor_copy(out=al2_sb[:], in_=psal2[:])
            # transpose -> [k, i], softmax along free dim (i)
            psal2t = pssm.tile([N, N], F32, tag="ps")
            nc.tensor.transpose(out=psal2t[:], in_=al2_sb[:], identity=ident32[:N, :N])
            negmx = small.tile([N, 1], F32, tag="negmx")
            nc.vector.tensor_reduce(
                out=negmx[:], in_=psal2t[:], axis=mybir.AxisListType.X,
                op=mybir.AluOpType.max, negate=True,
            )
            sm_sb = small.tile([N, N], F32, tag="smexp")
            ssum = small.tile([N, 1], F32, tag="ssum")
            nc.scalar.activation(
                out=sm_sb[:], in_=psal2t[:],
                func=mybir.ActivationFunctionType.Exp,
                bias=negmx[:], scale=1.0, accum_out=ssum[:],
            )
            rsum = small.tile([N, 1], F32, tag="rsum")
            nc.vector.reciprocal(out=rsum[:], in_=ssum[:])
            al3t_sb = small.tile([N, N], F32R, tag="al3t")
            nc.vector.tensor_scalar(
                out=al3t_sb[:], in0=sm_sb[:], scalar1=rsum[:], scalar2=None,
                op0=mybir.AluOpType.mult,
            )
            # out[i, d] = sum_k alpha3[i, k] lin[k, d]
            pso = psout.tile([N, D], F32, tag="pslin")
            for nch in range(2):
                sl = slice(nch * 512, (nch + 1) * 512)
                nc.tensor.matmul(
                    out=pso[:, sl], lhsT=al3t_sb[:], rhs=lin_sb[b][:, sl],
                    start=True, stop=True,
                )
            o_sb = outp.tile([N, D], F32, tag="osb")
            nc.vector.tensor_copy(out=o_sb[:], in_=pso[:])
            nc.sync.dma_start(out=out_d.ap()[b * N:(b + 1) * N, :], in_=o_sb[:])


def _prep_inputs(feature, graph, W0, W1, bias, dp_Wa, dp_ba, dp_Wb, dp_bb):
    feature = np.ascontiguousarray(np.asarray(feature, dtype=np.float32))
    graph = np.asarray(graph)
    bias = np.ascontiguousarray(np.asarray(bias, dtype=np.float32))
    wct = np.ascontiguousarray(np.asarray(W0, np.float32).T
                               + np.asarray(W1, np.float32).T)
    wat = np.ascontiguousarray(np.asarray(dp_Wa, np.float32).T)
    wbt = np.ascontiguousarray(np.asarray(dp_Wb, np.float32).T)
    ba = np.ascontiguousarray(np.asarray(dp_ba, np.float32))
    bb = np.ascontiguousarray(np.asarray(dp_bb, np.float32))
    g16 = graph.astype(np.float16)  # labels < 2048: exact in fp16
    j = np.arange(N)
    ltmask = np.ascontiguousarray(
        (j[None, :] < j[:, None]).astype(np.float16).reshape(-1))
    id32 = np.eye(128, dtype=np.float32)
    id16 = np.eye(128, dtype=np.float16)

    in_maps = []
    for c in range(NCORES):
        bs = slice(c * BPC, (c + 1) * BPC)
        in_maps.append({
            "x": np.ascontiguousarray(feature[bs].reshape(R, D)),
            "id32": id32,
            "id16": id16,
            "g16": np.ascontiguousarray(g16[bs].reshape(R, N)),
            "ltmask": ltmask,
            "wct": wct,
            "wat": wat,
            "wbt": wbt,
            "bias": bias,
            "ba": ba,
            "bb": bb,
        })
    return in_maps


def get_program():
    if "nc" not in _CACHE:
        _CACHE["nc"] = _build_program()
    return _CACHE["nc"]


def kernel(feature, graph, W0, W1, bias, dp_Wa, dp_ba, dp_Wb, dp_bb,
           get_alpha=0, **_ignored):
    from concourse.bass_utils import run_bass_kernel_spmd

    nc = get_program()
    in_maps = _prep_inputs(feature, graph, W0, W1, bias, dp_Wa, dp_ba,
                           dp_Wb, dp_bb)
    res = run_bass_kernel_spmd(nc, in_maps, list(range(NCORES)))
    out = np.concatenate(
        [res.results[c]["out"].reshape(BPC, N, D) for c in range(NCORES)], axis=0
    )
    return out


# revision 22
# speedup vs baseline: 1.6741x; 1.1456x over previous
"""Trainium2 Bass kernel for CorrelatedGraphConv.

Reference computation (per batch b, N=100 rows, D=1024, L=2000 labels):
    adj   = (graph != 0)
    lin   = x + x@W0.T + x@W1.T + sum_j bias[graph[:, j]]
    a     = x@Wa.T + ba ; bvec = x@Wb.T + bb
    alpha = relu(a @ bvec.T)
    alpha = softmax(adj @ alpha, axis=0)   # over rows i
    out   = alpha @ lin

Strategy: data-parallel over batch across 8 cores (2 batches/core).
The label-gather `sum_j bias[g[i,j]]` is computed as `C @ bias` where
C[i, l] = #{j : g[i,j] == l} is built on-chip with gpsimd.local_scatter
(per-partition indexed scatter); duplicate labels within a row are
pre-combined on DVE (count via self-equality matmul-free compare, only
the first occurrence scatters the total count).
"""

import numpy as np

import concourse.bass as bass
import concourse.mybir as mybir
import concourse.tile as tile
from concourse import bacc, library_config

F32 = mybir.dt.float32
F32R = mybir.dt.float32r
F16 = mybir.dt.float16
I16 = mybir.dt.int16

B, N, D, L = 16, 100, 1024, 2000
NCORES = 8
BPC = B // NCORES          # batches per core
R = BPC * N                # rows per core (200)
DT = D // 128              # 8 d-tiles
LT_TILES = (L + 127) // 128  # 16 label tiles (last is 80)
LPAD = 2048                  # padded label rows in scatter tables
ESC = 256                    # scatter element width (fp16) = 512B rows
NCALL = 5                    # scatter calls (one table each)
TPC = R * N // NCALL         # tokens per call (10000)
SCHUNK = (TPC + 127) // 128  # source chunks per call (79)

_CACHE = {}


def _bcast3(ap, mid, inner, mode):
    """[P, F] AP -> [P, mid, inner] broadcast view.

    mode 'j':  out[p, a, b] = ap[p, a]   (inner broadcast)
    mode 'jp': out[p, a, b] = ap[p, b]   (middle broadcast)
    """
    (pstep, pcount), (fstep, fcount) = ap.ap[0], ap.ap[1]
    if mode == "j":
        assert fcount == mid
        new = [[pstep, pcount], [fstep, mid], [0, inner]]
    else:
        assert fcount == inner
        new = [[pstep, pcount], [0, mid], [fstep, inner]]
    return bass.AP(tensor=ap.tensor, offset=ap.offset, ap=new)


def _pbcast(ap, p):
    """[1, ...] AP -> [p, ...] partition-broadcast view."""
    new = [[0, p]] + [list(d) for d in ap.ap[1:]]
    return bass.AP(tensor=ap.tensor, offset=ap.offset, ap=new)


def _build_program():
    nc = bacc.Bacc("TRN2", target_bir_lowering=False, debug=False,
                   num_devices=NCORES)

    x_d = nc.declare_dram_parameter("x", [R, D], F32, isOutput=False)
    id32_d = nc.declare_dram_parameter("id32", [128, 128], F32, isOutput=False)
    id16_d = nc.declare_dram_parameter("id16", [128, 128], F16, isOutput=False)
    g_d = nc.declare_dram_parameter("g16", [R, N], F16, isOutput=False)
    lt_d = nc.declare_dram_parameter("ltmask", [N * N], F16, isOutput=False)
    wct_d = nc.declare_dram_parameter("wct", [D, D], F32R, isOutput=False)
    wat_d = nc.declare_dram_parameter("wat", [D, D], F32R, isOutput=False)
    wbt_d = nc.declare_dram_parameter("wbt", [D, D], F32R, isOutput=False)
    bias_d = nc.declare_dram_parameter("bias", [L, D], F32R, isOutput=False)
    ba_d = nc.declare_dram_parameter("ba", [D], F32, isOutput=False)
    bb_d = nc.declare_dram_parameter("bb", [D], F32, isOutput=False)
    out_d = nc.declare_dram_parameter("out", [R, D], F32, isOutput=True)

    with tile.TileContext(nc) as tc:
        _emit(tc, x_d, id32_d, id16_d, g_d, lt_d, wct_d, wat_d, wbt_d,
              bias_d, ba_d, bb_d, out_d)
    nc.compile()
    return nc


def _emit(tc, x_d, id32_d, id16_d, g_d, lt_d, wct_d, wat_d, wbt_d, bias_d,
          ba_d, bb_d, out_d):
    nc = tc.nc
    import contextlib

    ctx = contextlib.ExitStack()
    with ctx:
        const = ctx.enter_context(tc.tile_pool(name="const", bufs=1))
        gpool = ctx.enter_context(tc.tile_pool(name="gtiles", bufs=1))
        xpool = ctx.enter_context(tc.tile_pool(name="xtiles", bufs=1))
        xt = ctx.enter_context(tc.tile_pool(name="xt", bufs=1))
        abp = ctx.enter_context(tc.tile_pool(name="abt", bufs=1))
        linp = ctx.enter_context(tc.tile_pool(name="lin", bufs=1))
        cb = ctx.enter_context(tc.tile_pool(name="cbuild", bufs=1))
        cpool = ctx.enter_context(tc.tile_pool(name="cmat", bufs=1))
        ctp = ctx.enter_context(tc.tile_pool(name="ctmat", bufs=1))
        wstream = ctx.enter_context(tc.tile_pool(name="wstream", bufs=4))
        bstream = ctx.enter_context(tc.tile_pool(name="bstream", bufs=4))
        small = ctx.enter_context(tc.tile_pool(name="small", bufs=2))
        outp = ctx.enter_context(tc.tile_pool(name="outs", bufs=2))
        pst = ctx.enter_context(tc.tile_pool(name="pst", bufs=3, space="PSUM"))
        psa = pst
        pslin = ctx.enter_context(tc.tile_pool(name="pslin", bufs=2, space="PSUM"))
        pssm = pst
        psout = pslin

        # ---- constants ----
        nc.gpsimd.load_library(library_config.local_scatter)
        ident32 = const.tile([128, 128], F32)
        nc.sync.dma_start(out=ident32[:], in_=id32_d.ap())
        ident16 = const.tile([128, 128], F16)
        nc.sync.dma_start(out=ident16[:], in_=id16_d.ap())
        lt_sb = const.tile([128, N * N], F16)
        lt_ap = lt_d.ap()
        nc.sync.dma_start(
            out=lt_sb[:],
            in_=bass.AP(tensor=lt_ap.tensor, offset=lt_ap.offset,
                        ap=[[0, 128], [1, N * N]]),
        )
        ba_sb = const.tile([128, DT], F32)
        nc.sync.dma_start(out=ba_sb[:], in_=ba_d.ap().rearrange("(t p) -> p t", p=128))
        bb_sb = const.tile([128, DT], F32)
        nc.sync.dma_start(out=bb_sb[:], in_=bb_d.ap().rearrange("(t p) -> p t", p=128))

        # ---- load x (per-batch row tiles) and build X_T [din, r] ----
        xg = x_d.ap()
        xb = []
        for b in range(BPC):
            t = xpool.tile([N, D], F32, tag=f"xb{b}")
            nc.sync.dma_start(out=t[:], in_=xg[b * N:(b + 1) * N, :])
            xb.append(t)

        RP = 256  # padded row dim for full-rate f32r matmul
        xt_sb = xt.tile([128, DT, RP], F32R)
        nc.vector.memset(xt_sb[:, :, R:RP].bitcast(F32), 0.0)
        for b in range(BPC):
            for dk in range(DT):
                pt = pst.tile([128, N], F32, tag="ps")
                nc.tensor.transpose(
                    out=pt[:],
                    in_=xb[b][:, dk * 128:(dk + 1) * 128],
                    identity=ident32[:N, :N],
                )
                nc.vector.tensor_copy(
                    out=xt_sb[:, dk, b * N:(b + 1) * N], in_=pt[:]
                )

        # ---- graph tiles ----
        gg = g_d.ap()
        gf0 = gpool.tile([128, N], F16, tag="gf0")
        nc.sync.dma_start(out=gf0[:], in_=gg[0:128, :])
        gf1 = gpool.tile([80, N], F16, tag="gf1")
        nc.sync.dma_start(out=gf1[:R - 128, :], in_=gg[128:R, :])
        gb = []
        for b in range(BPC):
            t = gpool.tile([N, N], F16, tag=f"gb{b}")
            nc.sync.dma_start(out=t[:], in_=gg[b * N:(b + 1) * N, :])
            gb.append(t)

        # ---- histogram C[r, l] (fp16) via local_scatter, two row tiles ----
        c_tiles = []
        for ti, (gf, rows) in enumerate(((gf0, 128), (gf1, R - 128))):
            meq = cb.tile([128, N, N], F16, tag="meq")
            nc.vector.tensor_tensor(
                out=meq[:rows],
                in0=_bcast3(gf[:rows, :], N, N, "j"),
                in1=_bcast3(gf[:rows, :], N, N, "jp"),
                op=mybir.AluOpType.is_equal,
            )
            cnt32 = cb.tile([128, N], F32, tag="cnt32")
            nc.vector.tensor_reduce(
                out=cnt32[:rows], in_=meq[:rows], axis=mybir.AxisListType.X,
                op=mybir.AluOpType.add,
            )
            lt_full = lt_sb[:]
            nc.vector.tensor_tensor(
                out=meq[:rows],
                in0=meq[:rows],
                in1=bass.AP(tensor=lt_full.tensor, offset=lt_full.offset,
                            ap=[[lt_full.ap[0][0], rows], [N, N], [1, N]]),
                op=mybir.AluOpType.mult,
            )
            rank32 = cb.tile([128, N], F32, tag="rank32")
            nc.vector.tensor_reduce(
                out=rank32[:rows], in_=meq[:rows], axis=mybir.AxisListType.X,
                op=mybir.AluOpType.add,
            )
            fo = cb.tile([128, N], F16, tag="fo")
            nc.vector.tensor_scalar(
                out=fo[:rows], in0=rank32[:rows], scalar1=0.0, scalar2=None,
                op0=mybir.AluOpType.is_equal,
            )
            gp1 = cb.tile([128, N], F16, tag="gp1")
            nc.vector.tensor_scalar(
                out=gp1[:rows], in0=gf[:rows, :], scalar1=1.0, scalar2=None,
                op0=mybir.AluOpType.add,
            )
            idxf = cb.tile([128, N], F16, tag="idxf")
            nc.vector.scalar_tensor_tensor(
                out=idxf[:rows], in0=fo[:rows], scalar=-1.0, in1=gp1[:rows],
                op0=mybir.AluOpType.bypass, op1=mybir.AluOpType.mult,
            )
            nc.vector.tensor_scalar(
                out=idxf[:rows], in0=idxf[:rows], scalar1=-1.0, scalar2=None,
                op0=mybir.AluOpType.add,
            )
            nch = 128 if ti == 0 else 80
            idx16 = cb.tile([128, N], I16, tag="idx16")
            cnt16 = cb.tile([128, N], F16, tag="cnt16")
            if rows < nch:
                nc.vector.memset(idx16[:nch, :], -1)
                nc.vector.memset(cnt16[:nch, :], 0.0)
            nc.vector.tensor_copy(out=idx16[:rows], in_=idxf[:rows])
            nc.vector.tensor_copy(out=cnt16[:rows], in_=cnt32[:rows])
            cmat = cpool.tile([nch, L], F16, tag=f"c{ti}")
            nc.gpsimd.local_scatter(
                out_ap=cmat[:],
                data_ap=cnt16[:nch],
                idxs_ap=idx16[:nch],
                channels=nch,
                num_elems=L,
                num_idxs=N,
            )
            c_tiles.append((cmat, rows))

        # ---- A_T / B_T = (x @ Wa.T).T etc, [dout, r] orientation ----
        # weights streamed as column panels WqT[:, dt_i*128:(dt_i+1)*128]
        # reshaped to [128(din), DT, 128(dout)] so each panel is consumed
        # by its 8 matmuls then released.
        at_sb = abp.tile([128, DT, R], F32R, tag="at")
        bt_sb = abp.tile([128, DT, R], F32R, tag="bt")
        for w_d, bias_col, dst in (
            (wat_d, ba_sb, at_sb), (wbt_d, bb_sb, bt_sb)
        ):
            for dt_i in range(DT):
                panel = wstream.tile([128, DT, 128], F32R, tag="wpanel")
                nc.sync.dma_start(
                    out=panel[:],
                    in_=w_d.ap()[:, dt_i * 128:(dt_i + 1) * 128].rearrange(
                        "(t p) c -> p t c", p=128
                    ),
                )
                ps = psa.tile([128, RP], F32, tag="ps")
                for dk in range(DT):
                    nc.tensor.matmul(
                        out=ps[:],
                        lhsT=panel[:, dk, :],
                        rhs=xt_sb[:, dk, :],
                        start=(dk == 0),
                        stop=(dk == DT - 1),
                    )
                # evacuate on ScalarE: DVE's queue is busy with the
                # histogram build and would stall the PE psum rotation
                nc.scalar.activation(
                    out=dst[:, dt_i, :], in_=ps[:, 0:R],
                    func=mybir.ActivationFunctionType.Identity,
                    bias=bias_col[:, dt_i:dt_i + 1], scale=1.0,
                )

        # ---- C^T tiles [l, r] via PE transpose ----
        ct_sb = ctp.tile([128, LT_TILES, R], F32R)
        for lc in range(LT_TILES):
            cs = min(128, L - lc * 128)
            col = 0
            for (cmat, rows) in c_tiles:
                pt = pst.tile([128, 128], F16, tag="ps")
                nc.tensor.transpose(
                    out=pt[:cs, :cmat.shape[0]],
                    in_=cmat[:, lc * 128:lc * 128 + cs],
                    identity=ident16[:cmat.shape[0], :cmat.shape[0]],
                )
                nc.vector.tensor_copy(
                    out=ct_sb[:cs, lc, col:col + rows], in_=pt[:cs, :rows]
                )
                col += rows

        # ---- LIN[r, d] = x + x@Wc.T + C@bias  (per batch psum) ----
        # stream WcT row tiles / bias label tiles; dk/lc outer so each
        # streamed tile is consumed then released.
        lin_ps = []
        for b in range(BPC):
            lp = pslin.tile([N, D], F32, tag="pslin")
            lin_ps.append(lp)
        for dk in range(DT):
            wt = wstream.tile([128, D], F32R, tag="wpanel")
            nc.sync.dma_start(out=wt[:], in_=wct_d.ap()[dk * 128:(dk + 1) * 128, :])
            for b in range(BPC):
                for nch in range(2):
                    sl = slice(nch * 512, (nch + 1) * 512)
                    nc.tensor.matmul(
                        out=lin_ps[b][:, sl],
                        lhsT=xt_sb[:, dk, b * N:(b + 1) * N],
                        rhs=wt[:, sl],
                        start=(dk == 0),
                        stop=False,
                    )
        for lc in range(LT_TILES):
            cs = min(128, L - lc * 128)
            btile = bstream.tile([128, D], F32R, tag="btile")
            nc.sync.dma_start(out=btile[:cs], in_=bias_d.ap()[lc * 128:lc * 128 + cs, :])
            for b in range(BPC):
                for nch in range(2):
                    sl = slice(nch * 512, (nch + 1) * 512)
                    nc.tensor.matmul(
                        out=lin_ps[b][:, sl],
                        lhsT=ct_sb[:cs, lc, b * N:(b + 1) * N],
                        rhs=btile[:cs, sl],
                        start=False,
                        stop=(lc == LT_TILES - 1),
                    )
        lin_sb = []
        for b in range(BPC):
            t = linp.tile([N, D], F32R, tag=f"lin{b}")
            nc.vector.tensor_add(t[:], lin_ps[b][:], xb[b][:])
            lin_sb.append(t)

        # ---- per-batch attention ----
        for b in range(BPC):
            rsl = slice(b * N, (b + 1) * N)
            # alpha_raw[i, j] = a_i . b_j  -> relu
            psal = pssm.tile([N, N], F32, tag="ps")
            for dk in range(DT):
                nc.tensor.matmul(
                    out=psal[:],
                    lhsT=at_sb[:, dk, rsl],
                    rhs=bt_sb[:, dk, rsl],
                    start=(dk == 0),
                    stop=(dk == DT - 1),
                )
            alpha_sb = small.tile([N, N], F32R, tag="alpha")
            nc.vector.tensor_scalar(
                out=alpha_sb[:], in0=psal[:], scalar1=0.0, scalar2=None,
                op0=mybir.AluOpType.max,
            )
            # adjT[j, i] = (g[i, j] != 0)
            psgt = pst.tile([N, N], F16, tag="ps")
            nc.tensor.transpose(out=psgt[:], in_=gb[b][:], identity=ident16[:N, :N])
            adjt_sb = small.tile([N, N], F32R, tag="adjt")
            nc.vector.tensor_scalar(
                out=adjt_sb[:], in0=psgt[:], scalar1=0.0, scalar2=None,
                op0=mybir.AluOpType.not_equal,
            )
            # alpha2[i, k] = sum_j adj[i, j] alpha[j, k]
            psal2 = pssm.tile([N, N], F32, tag="ps")
            nc.tensor.matmul(
                out=psal2[:], lhsT=adjt_sb[:], rhs=alpha_sb[:],
                start=True, stop=True,
            )
            al2_sb = small.tile([N, N], F32, tag="al2")
            nc.vector.tensor_copy(out=al2_sb[:], in_=psal2[:])
            # transpose -> [k, i], softmax along free dim (i)
            psal2t = pssm.tile([N, N], F32, tag="ps")
            nc.tensor.transpose(out=psal2t[:], in_=al2_sb[:], identity=ident32[:N, :N])
            negmx = small.tile([N, 1], F32, tag="negmx")
            nc.vector.tensor_reduce(
                out=negmx[:], in_=psal2t[:], axis=mybir.AxisListType.X,
                op=mybir.AluOpType.max, negate=True,
            )
            sm_sb = small.tile([N, N], F32, tag="smexp")
            ssum = small.tile([N, 1], F32, tag="ssum")
            nc.scalar.activation(
                out=sm_sb[:], in_=psal2t[:],
                func=mybir.ActivationFunctionType.Exp,
                bias=negmx[:], scale=1.0, accum_out=ssum[:],
            )
            rsum = small.tile([N, 1], F32, tag="rsum")
            nc.vector.reciprocal(out=rsum[:], in_=ssum[:])
            al3t_sb = small.tile([N, N], F32R, tag="al3t")
            nc.vector.tensor_scalar(
                out=al3t_sb[:], in0=sm_sb[:], scalar1=rsum[:], scalar2=None,
                op0=mybir.AluOpType.mult,
            )
            # out[i, d] = sum_k alpha3[i, k] lin[k, d]
            pso = psout.tile([N, D], F32, tag="pslin")
            for nch in range(2):
                sl = slice(nch * 512, (nch + 1) * 512)
                nc.tensor.matmul(
                    out=pso[:, sl], lhsT=al3t_sb[:], rhs=lin_sb[b][:, sl],
                    start=True, stop=True,
                )
            o_sb = outp.tile([N, D], F32, tag="osb")
            nc.vector.tensor_copy(out=o_sb[:], in_=pso[:])
            nc.sync.dma_start(out=out_d.ap()[b * N:(b + 1) * N, :], in_=o_sb[:])


def _prep_inputs(feature, graph, W0, W1, bias, dp_Wa, dp_ba, dp_Wb, dp_bb):
    feature = np.ascontiguousarray(np.asarray(feature, dtype=np.float32))
    graph = np.asarray(graph)
    bias = np.ascontiguousarray(np.asarray(bias, dtype=np.float32))
    wct = np.ascontiguousarray(np.asarray(W0, np.float32).T
                               + np.asarray(W1, np.float32).T)
    wat = np.ascontiguousarray(np.asarray(dp_Wa, np.float32).T)
    wbt = np.ascontiguousarray(np.asarray(dp_Wb, np.float32).T)
    ba = np.ascontiguousarray(np.asarray(dp_ba, np.float32))
    bb = np.ascontiguousarray(np.asarray(dp_bb, np.float32))
    g16 = graph.astype(np.float16)  # labels < 2048: exact in fp16
    j = np.arange(N)
    ltmask = np.ascontiguousarray(
        (j[None, :] < j[:, None]).astype(np.float16).reshape(-1))
    id32 = np.eye(128, dtype=np.float32)
    id16 = np.eye(128, dtype=np.float16)

    in_maps = []
    for c in range(NCORES):
        bs = slice(c * BPC, (c + 1) * BPC)
        in_maps.append({
            "x": np.ascontiguousarray(feature[bs].reshape(R, D)),
            "id32": id32,
            "id16": id16,
            "g16": np.ascontiguousarray(g16[bs].reshape(R, N)),
            "ltmask": ltmask,
            "wct": wct,
            "wat": wat,
            "wbt": wbt,
            "bias": bias,
            "ba": ba,
            "bb": bb,
        })
    return in_maps


def get_program():
    if "nc" not in _CACHE:
        _CACHE["nc"] = _build_program()
    return _CACHE["nc"]


def kernel(feature, graph, W0, W1, bias, dp_Wa, dp_ba, dp_Wb, dp_bb,
           get_alpha=0, **_ignored):
    from concourse.bass_utils import run_bass_kernel_spmd

    nc = get_program()
    in_maps = _prep_inputs(feature, graph, W0, W1, bias, dp_Wa, dp_ba,
                           dp_Wb, dp_bb)
    res = run_bass_kernel_spmd(nc, in_maps, list(range(NCORES)))
    out = np.concatenate(
        [res.results[c]["out"].reshape(BPC, N, D) for c in range(NCORES)], axis=0
    )
    return out


# revision 23
# speedup vs baseline: 1.8857x; 1.1264x over previous
"""Trainium2 Bass kernel for CorrelatedGraphConv.

Reference computation (per batch b, N=100 rows, D=1024, L=2000 labels):
    adj   = (graph != 0)
    lin   = x + x@W0.T + x@W1.T + sum_j bias[graph[:, j]]
    a     = x@Wa.T + ba ; bvec = x@Wb.T + bb
    alpha = relu(a @ bvec.T)
    alpha = softmax(adj @ alpha, axis=0)   # over rows i
    out   = alpha @ lin

Strategy: data-parallel over batch across 8 cores (2 batches/core).
The label-gather `sum_j bias[g[i,j]]` is computed as `C @ bias` where
C[i, l] = #{j : g[i,j] == l} is built on-chip with gpsimd.local_scatter
(per-partition indexed scatter); duplicate labels within a row are
pre-combined on DVE (count via self-equality matmul-free compare, only
the first occurrence scatters the total count).
"""

import numpy as np

import concourse.bass as bass
import concourse.mybir as mybir
import concourse.tile as tile
from concourse import bacc, library_config

F32 = mybir.dt.float32
F32R = mybir.dt.float32r
F16 = mybir.dt.float16
I16 = mybir.dt.int16

B, N, D, L = 16, 100, 1024, 2000
NCORES = 8
BPC = B // NCORES          # batches per core
R = BPC * N                # rows per core (200)
DT = D // 128              # 8 d-tiles
LT_TILES = (L + 127) // 128  # 16 label tiles (last is 80)
LPAD = 2048                  # padded label rows in scatter tables
ESC = 256                    # scatter element width (fp16) = 512B rows
NCALL = 5                    # scatter calls (one table each)
TPC = R * N // NCALL         # tokens per call (10000)
SCHUNK = (TPC + 127) // 128  # source chunks per call (79)

_CACHE = {}


def _bcast3(ap, mid, inner, mode):
    """[P, F] AP -> [P, mid, inner] broadcast view.

    mode 'j':  out[p, a, b] = ap[p, a]   (inner broadcast)
    mode 'jp': out[p, a, b] = ap[p, b]   (middle broadcast)
    """
    (pstep, pcount), (fstep, fcount) = ap.ap[0], ap.ap[1]
    if mode == "j":
        assert fcount == mid
        new = [[pstep, pcount], [fstep, mid], [0, inner]]
    else:
        assert fcount == inner
        new = [[pstep, pcount], [0, mid], [fstep, inner]]
    return bass.AP(tensor=ap.tensor, offset=ap.offset, ap=new)


def _pbcast(ap, p):
    """[1, ...] AP -> [p, ...] partition-broadcast view."""
    new = [[0, p]] + [list(d) for d in ap.ap[1:]]
    return bass.AP(tensor=ap.tensor, offset=ap.offset, ap=new)


def _build_program():
    nc = bacc.Bacc("TRN2", target_bir_lowering=False, debug=False,
                   num_devices=NCORES)

    x_d = nc.declare_dram_parameter("x", [R, D], F32, isOutput=False)
    id32_d = nc.declare_dram_parameter("id32", [128, 128], F32, isOutput=False)
    id16_d = nc.declare_dram_parameter("id16", [128, 128], F16, isOutput=False)
    g_d = nc.declare_dram_parameter("g16", [R, N], F16, isOutput=False)
    lt_d = nc.declare_dram_parameter("ltmask", [N * N], F16, isOutput=False)
    wct_d = nc.declare_dram_parameter("wct", [D, D], F32R, isOutput=False)
    wat_d = nc.declare_dram_parameter("wat", [D, D], F32R, isOutput=False)
    wbt_d = nc.declare_dram_parameter("wbt", [D, D], F32R, isOutput=False)
    bias_d = nc.declare_dram_parameter("bias", [L, D], F32R, isOutput=False)
    ba_d = nc.declare_dram_parameter("ba", [D], F32, isOutput=False)
    bb_d = nc.declare_dram_parameter("bb", [D], F32, isOutput=False)
    out_d = nc.declare_dram_parameter("out", [R, D], F32, isOutput=True)

    with tile.TileContext(nc) as tc:
        _emit(tc, x_d, id32_d, id16_d, g_d, lt_d, wct_d, wat_d, wbt_d,
              bias_d, ba_d, bb_d, out_d)
    nc.compile()
    return nc


def _emit(tc, x_d, id32_d, id16_d, g_d, lt_d, wct_d, wat_d, wbt_d, bias_d,
          ba_d, bb_d, out_d):
    nc = tc.nc
    import contextlib

    ctx = contextlib.ExitStack()
    with ctx:
        const = ctx.enter_context(tc.tile_pool(name="const", bufs=1))
        gpool = ctx.enter_context(tc.tile_pool(name="gtiles", bufs=1))
        xpool = ctx.enter_context(tc.tile_pool(name="xtiles", bufs=1))
        xt = ctx.enter_context(tc.tile_pool(name="xt", bufs=1))
        abp = ctx.enter_context(tc.tile_pool(name="abt", bufs=1))
        linp = ctx.enter_context(tc.tile_pool(name="lin", bufs=1))
        cb = ctx.enter_context(tc.tile_pool(name="cbuild", bufs=1))
        cpool = ctx.enter_context(tc.tile_pool(name="cmat", bufs=2))
        ctp = ctx.enter_context(tc.tile_pool(name="ctmat", bufs=2))
        wstream = ctx.enter_context(tc.tile_pool(name="wstream", bufs=4))
        bstream = ctx.enter_context(tc.tile_pool(name="bstream", bufs=4))
        small = ctx.enter_context(tc.tile_pool(name="small", bufs=2))
        outp = ctx.enter_context(tc.tile_pool(name="outs", bufs=2))
        pst = ctx.enter_context(tc.tile_pool(name="pst", bufs=3, space="PSUM"))
        psa = pst
        pslin = ctx.enter_context(tc.tile_pool(name="pslin", bufs=2, space="PSUM"))
        pssm = pst
        psout = pslin

        nc.gpsimd.load_library(library_config.local_scatter)

        # ---- input DMAs: x and g first (gate the PE/DVE pipelines) ----
        xg = x_d.ap()
        xb = []
        for b in range(BPC):
            t = xpool.tile([N, D], F32, tag=f"xb{b}")
            nc.sync.dma_start(out=t[:], in_=xg[b * N:(b + 1) * N, :])
            xb.append(t)
        ident32 = const.tile([128, 128], F32)
        nc.sync.dma_start(out=ident32[:], in_=id32_d.ap())
        ident16 = const.tile([128, 128], F16)
        nc.sync.dma_start(out=ident16[:], in_=id16_d.ap())
        gg = g_d.ap()
        gb = []
        for b in range(BPC):
            t = gpool.tile([N, N], F16, tag=f"gb{b}")
            nc.sync.dma_start(out=t[:], in_=gg[b * N:(b + 1) * N, :])
            gb.append(t)
        # LT mask broadcast to all partitions, split into 4 DMAs (queue spread)
        lt_sb = const.tile([128, N * N], F16)
        lt_ap = lt_d.ap()
        for q in range(4):
            nc.sync.dma_start(
                out=lt_sb[q * 32:(q + 1) * 32, :],
                in_=bass.AP(tensor=lt_ap.tensor, offset=lt_ap.offset,
                            ap=[[0, 32], [1, N * N]]),
            )
        ba_sb = const.tile([128, DT], F32)
        nc.sync.dma_start(out=ba_sb[:], in_=ba_d.ap().rearrange("(t p) -> p t", p=128))
        bb_sb = const.tile([128, DT], F32)
        nc.sync.dma_start(out=bb_sb[:], in_=bb_d.ap().rearrange("(t p) -> p t", p=128))

        # ---- X_T [din, r] via PE transpose (padded to 256 for f32r rate) ----
        RP = 256
        xt_sb = xt.tile([128, DT, RP], F32R)
        nc.vector.memset(xt_sb[:, :, R:RP].bitcast(F32), 0.0)
        for b in range(BPC):
            for dk in range(DT):
                pt = pst.tile([128, N], F32, tag="ps")
                nc.tensor.transpose(
                    out=pt[:],
                    in_=xb[b][:, dk * 128:(dk + 1) * 128],
                    identity=ident32[:N, :N],
                )
                nc.vector.tensor_copy(
                    out=xt_sb[:, dk, b * N:(b + 1) * N], in_=pt[:]
                )

        # ---- A_T / B_T (PE + ScalarE evac; weights as column panels) ----
        at_sb = abp.tile([128, DT, R], F32R, tag="at")
        bt_sb = abp.tile([128, DT, R], F32R, tag="bt")
        for w_d, bias_col, dst in (
            (wat_d, ba_sb, at_sb), (wbt_d, bb_sb, bt_sb)
        ):
            for dt_i in range(DT):
                panel = wstream.tile([128, DT, 128], F32R, tag="wpanel")
                nc.sync.dma_start(
                    out=panel[:],
                    in_=w_d.ap()[:, dt_i * 128:(dt_i + 1) * 128].rearrange(
                        "(t p) c -> p t c", p=128
                    ),
                )
                ps = psa.tile([128, RP], F32, tag="ps")
                for dk in range(DT):
                    nc.tensor.matmul(
                        out=ps[:],
                        lhsT=panel[:, dk, :],
                        rhs=xt_sb[:, dk, :],
                        start=(dk == 0),
                        stop=(dk == DT - 1),
                    )
                nc.scalar.activation(
                    out=dst[:, dt_i, :], in_=ps[:, 0:R],
                    func=mybir.ActivationFunctionType.Identity,
                    bias=bias_col[:, dt_i:dt_i + 1], scale=1.0,
                )

        # ---- LIN psums: x @ Wc.T part (counts part accumulates later) ----
        lin_ps = []
        for b in range(BPC):
            lp = pslin.tile([N, D], F32, tag="pslin")
            lin_ps.append(lp)
        for dk in range(DT):
            wt = wstream.tile([128, D], F32R, tag="wpanel")
            nc.sync.dma_start(out=wt[:], in_=wct_d.ap()[dk * 128:(dk + 1) * 128, :])
            for b in range(BPC):
                for nch in range(2):
                    sl = slice(nch * 512, (nch + 1) * 512)
                    nc.tensor.matmul(
                        out=lin_ps[b][:, sl],
                        lhsT=xt_sb[:, dk, b * N:(b + 1) * N],
                        rhs=wt[:, sl],
                        start=(dk == 0),
                        stop=False,
                    )
        bias_tiles = []
        for lc in range(LT_TILES):
            cs = min(128, L - lc * 128)
            btile = bstream.tile([128, D], F32R, tag="btile")
            nc.sync.dma_start(out=btile[:cs],
                              in_=bias_d.ap()[lc * 128:lc * 128 + cs, :])
            bias_tiles.append(btile)

        # ---- per-batch: histogram -> C^T -> counts matmul -> attention ----
        NCH = 112  # local_scatter channels covering 100 rows
        HALF = N // 2
        lt_full = lt_sb[:]
        for b in range(BPC):
            gf = gb[b]
            # meq[i, j, jp] = (g[i,j] == g[i,jp])
            meq = cb.tile([NCH, N, N], F16, tag="meq")
            nc.vector.tensor_tensor(
                out=meq[:N],
                in0=_bcast3(gf[:], N, N, "j"),
                in1=_bcast3(gf[:], N, N, "jp"),
                op=mybir.AluOpType.is_equal,
            )
            # count = sum_jp meq : fold 100->50->25 (2x tensor_tensor), then reduce
            cf1 = cb.tile([NCH, N, HALF], F16, tag="cf1")
            nc.vector.tensor_tensor(
                out=cf1[:N], in0=meq[:N, :, 0:HALF], in1=meq[:N, :, HALF:N],
                op=mybir.AluOpType.add,
            )
            cf2 = cb.tile([NCH, N, HALF // 2], F16, tag="cf2")
            nc.vector.tensor_tensor(
                out=cf2[:N], in0=cf1[:N, :, 0:HALF // 2], in1=cf1[:N, :, HALF // 2:HALF],
                op=mybir.AluOpType.add,
            )
            cnt32 = cb.tile([NCH, N], F32, tag="cnt32")
            nc.vector.tensor_reduce(
                out=cnt32[:N], in_=cf2[:N], axis=mybir.AxisListType.X,
                op=mybir.AluOpType.add,
            )
            # rank = sum_{jp<j} meq : mask in place, fold, reduce
            nc.vector.tensor_tensor(
                out=meq[:N],
                in0=meq[:N],
                in1=bass.AP(tensor=lt_full.tensor, offset=lt_full.offset,
                            ap=[[lt_full.ap[0][0], N], [N, N], [1, N]]),
                op=mybir.AluOpType.mult,
            )
            nc.vector.tensor_tensor(
                out=cf1[:N], in0=meq[:N, :, 0:HALF], in1=meq[:N, :, HALF:N],
                op=mybir.AluOpType.add,
            )
            nc.vector.tensor_tensor(
                out=cf2[:N], in0=cf1[:N, :, 0:HALF // 2], in1=cf1[:N, :, HALF // 2:HALF],
                op=mybir.AluOpType.add,
            )
            rank32 = cb.tile([NCH, N], F32, tag="rank32")
            nc.vector.tensor_reduce(
                out=rank32[:N], in_=cf2[:N], axis=mybir.AxisListType.X,
                op=mybir.AluOpType.add,
            )
            # scatter idx: g where first occurrence else -1; data: count
            fo = cb.tile([NCH, N], F16, tag="fo")
            nc.vector.tensor_scalar(
                out=fo[:N], in0=rank32[:N], scalar1=0.0, scalar2=None,
                op0=mybir.AluOpType.is_equal,
            )
            gp1 = cb.tile([NCH, N], F16, tag="gp1")
            nc.vector.tensor_scalar(
                out=gp1[:N], in0=gf[:], scalar1=1.0, scalar2=None,
                op0=mybir.AluOpType.add,
            )
            idxf = cb.tile([NCH, N], F16, tag="idxf")
            nc.vector.tensor_tensor(
                out=idxf[:N], in0=fo[:N], in1=gp1[:N], op=mybir.AluOpType.mult,
            )
            nc.vector.tensor_scalar(
                out=idxf[:N], in0=idxf[:N], scalar1=-1.0, scalar2=None,
                op0=mybir.AluOpType.add,
            )
            idx16 = cb.tile([NCH, N], I16, tag="idx16")
            cnt16 = cb.tile([NCH, N], F16, tag="cnt16")
            nc.vector.memset(idx16[:NCH, :], -1)
            nc.vector.memset(cnt16[:NCH, :], 0.0)
            nc.vector.tensor_copy(out=idx16[:N], in_=idxf[:N])
            nc.vector.tensor_copy(out=cnt16[:N], in_=cnt32[:N])
            cmat = cpool.tile([NCH, L], F16, tag="cmat")
            nc.gpsimd.local_scatter(
                out_ap=cmat[:],
                data_ap=cnt16[:NCH],
                idxs_ap=idx16[:NCH],
                channels=NCH,
                num_elems=L,
                num_idxs=N,
            )
            # C^T tiles for this batch
            ct_sb = ctp.tile([128, LT_TILES, N], F32R, tag="ct")
            for lc in range(LT_TILES):
                cs = min(128, L - lc * 128)
                pt = pst.tile([128, N], F16, tag="ps")
                nc.tensor.transpose(
                    out=pt[:cs, :],
                    in_=cmat[:N, lc * 128:lc * 128 + cs],
                    identity=ident16[:N, :N],
                )
                nc.vector.tensor_copy(out=ct_sb[:cs, lc, :], in_=pt[:cs, :])
            # counts part of LIN
            for lc in range(LT_TILES):
                cs = min(128, L - lc * 128)
                for nch in range(2):
                    sl = slice(nch * 512, (nch + 1) * 512)
                    nc.tensor.matmul(
                        out=lin_ps[b][:, sl],
                        lhsT=ct_sb[:cs, lc, :],
                        rhs=bias_tiles[lc][:cs, sl],
                        start=False,
                        stop=(lc == LT_TILES - 1),
                    )
            lin_sb = linp.tile([N, D], F32R, tag=f"lin{b}")
            nc.vector.tensor_add(lin_sb[:], lin_ps[b][:], xb[b][:])

            # ---- attention for this batch ----
            rsl = slice(b * N, (b + 1) * N)
            psal = pssm.tile([N, N], F32, tag="ps")
            for dk in range(DT):
                nc.tensor.matmul(
                    out=psal[:],
                    lhsT=at_sb[:, dk, rsl],
                    rhs=bt_sb[:, dk, rsl],
                    start=(dk == 0),
                    stop=(dk == DT - 1),
                )
            alpha_sb = small.tile([N, N], F32R, tag="alpha")
            nc.scalar.activation(
                out=alpha_sb[:], in_=psal[:],
                func=mybir.ActivationFunctionType.Relu,
            )
            psgt = pst.tile([N, N], F16, tag="ps")
            nc.tensor.transpose(out=psgt[:], in_=gf[:], identity=ident16[:N, :N])
            adjt_sb = small.tile([N, N], F32R, tag="adjt")
            nc.vector.tensor_scalar(
                out=adjt_sb[:], in0=psgt[:], scalar1=0.0, scalar2=None,
                op0=mybir.AluOpType.not_equal,
            )
            psal2 = pssm.tile([N, N], F32, tag="ps")
            nc.tensor.matmul(
                out=psal2[:], lhsT=adjt_sb[:], rhs=alpha_sb[:],
                start=True, stop=True,
            )
            al2_sb = small.tile([N, N], F32, tag="al2")
            nc.scalar.activation(
                out=al2_sb[:], in_=psal2[:],
                func=mybir.ActivationFunctionType.Copy,
            )
            psal2t = pssm.tile([N, N], F32, tag="ps")
            nc.tensor.transpose(out=psal2t[:], in_=al2_sb[:], identity=ident32[:N, :N])
            negmx = small.tile([N, 1], F32, tag="negmx")
            nc.vector.tensor_reduce(
                out=negmx[:], in_=psal2t[:], axis=mybir.AxisListType.X,
                op=mybir.AluOpType.max, negate=True,
            )
            sm_sb = small.tile([N, N], F32, tag="smexp")
            ssum = small.tile([N, 1], F32, tag="ssum")
            nc.scalar.activation(
                out=sm_sb[:], in_=psal2t[:],
                func=mybir.ActivationFunctionType.Exp,
                bias=negmx[:], scale=1.0, accum_out=ssum[:],
            )
            rsum = small.tile([N, 1], F32, tag="rsum")
            nc.vector.reciprocal(out=rsum[:], in_=ssum[:])
            al3t_sb = small.tile([N, N], F32R, tag="al3t")
            nc.vector.tensor_scalar(
                out=al3t_sb[:], in0=sm_sb[:], scalar1=rsum[:], scalar2=None,
                op0=mybir.AluOpType.mult,
            )
            pso = psout.tile([N, D], F32, tag="pslin")
            for nch in range(2):
                sl = slice(nch * 512, (nch + 1) * 512)
                nc.tensor.matmul(
                    out=pso[:, sl], lhsT=al3t_sb[:], rhs=lin_sb[:, sl],
                    start=True, stop=True,
                )
            o_sb = outp.tile([N, D], F32, tag="osb")
            nc.vector.tensor_copy(out=o_sb[:], in_=pso[:])
            nc.sync.dma_start(out=out_d.ap()[b * N:(b + 1) * N, :], in_=o_sb[:])


def _prep_inputs(feature, graph, W0, W1, bias, dp_Wa, dp_ba, dp_Wb, dp_bb):
    feature = np.ascontiguousarray(np.asarray(feature, dtype=np.float32))
    graph = np.asarray(graph)
    bias = np.ascontiguousarray(np.asarray(bias, dtype=np.float32))
    wct = np.ascontiguousarray(np.asarray(W0, np.float32).T
                               + np.asarray(W1, np.float32).T)
    wat = np.ascontiguousarray(np.asarray(dp_Wa, np.float32).T)
    wbt = np.ascontiguousarray(np.asarray(dp_Wb, np.float32).T)
    ba = np.ascontiguousarray(np.asarray(dp_ba, np.float32))
    bb = np.ascontiguousarray(np.asarray(dp_bb, np.float32))
    g16 = graph.astype(np.float16)  # labels < 2048: exact in fp16
    j = np.arange(N)
    ltmask = np.ascontiguousarray(
        (j[None, :] < j[:, None]).astype(np.float16).reshape(-1))
    id32 = np.eye(128, dtype=np.float32)
    id16 = np.eye(128, dtype=np.float16)

    in_maps = []
    for c in range(NCORES):
        bs = slice(c * BPC, (c + 1) * BPC)
        in_maps.append({
            "x": np.ascontiguousarray(feature[bs].reshape(R, D)),
            "id32": id32,
            "id16": id16,
            "g16": np.ascontiguousarray(g16[bs].reshape(R, N)),
            "ltmask": ltmask,
            "wct": wct,
            "wat": wat,
            "wbt": wbt,
            "bias": bias,
            "ba": ba,
            "bb": bb,
        })
    return in_maps


def get_program():
    if "nc" not in _CACHE:
        _CACHE["nc"] = _build_program()
    return _CACHE["nc"]


def kernel(feature, graph, W0, W1, bias, dp_Wa, dp_ba, dp_Wb, dp_bb,
           get_alpha=0, **_ignored):
    from concourse.bass_utils import run_bass_kernel_spmd

    nc = get_program()
    in_maps = _prep_inputs(feature, graph, W0, W1, bias, dp_Wa, dp_ba,
                           dp_Wb, dp_bb)
    res = run_bass_kernel_spmd(nc, in_maps, list(range(NCORES)))
    out = np.concatenate(
        [res.results[c]["out"].reshape(BPC, N, D) for c in range(NCORES)], axis=0
    )
    return out
